# revision 12
# baseline (speedup 1.0000x reference)
"""Trainium2 Bass kernel for nn_CanadarmJacob (centroidal-dynamics jacobian).

Data-parallel over 8 NeuronCores; per core 32768 flat samples split into
NBLK=4 blocks of [P=128 partitions, F=64 free].  All per-sample quantities
live channel-major ([P, ch*F]) so every vector-op operand has a unit-stride
F-sized last dim -> DVE 2-byte fast modes apply.  Whole pipeline is bf16
scalar_tensor_tensor / tensor_scalar (InstTensorScalarPtr: 4x on DVE,
0.60-eff on Pool); ops are shaped so every access pattern canonicalizes to
<= 3 dims (stt verifier limit).  Only the 63 input floats/sample the
reference actually reads are shipped (com 21, link positions 21, jacobian
rows 0:3), packed host-side into one fused bf16 tensor.

Math (same validated algebra as the fp32 baseline):
  RP = C - P ;  MC = m_i*C ;  U[a,dd,i] = RP[a]*(MC[dd] | m_i)
  G = suffix_j(U) -> G[a,d,j], R[a,j] ;  rt = sum_i MC
  r = rt/M_tot - (0,0,beta) ;  T[a,j] = sum_d G[a,d,j]*J[d,j]
  trG ; rr = sum_a rt[a]R[a,j] ; rj = sum_a rt[a]J[a,j]
  u = trG - rr/M_tot ;  H_th = (DCUM + u)*J - T + (rj_b*R)/M_tot
  J_tw = J_j x R_j
H_s = K r r^T + diag(C1 - K|r|^2) = D + P_m with D = diag(C1) constant and
|P_m|/|D| <= ~4e-3, so first-order Neumann:  H_s^-1 ~= D^-1 - D^-1 P_m D^-1
  Y = Hth/C1 ;  s = sum_a r[a]Y[a,:] ;  q = |r|^2
  bot = w[a]*s - g[a]*Y ,  g = 1 + (K/C1[a]) q ,  w = (K/C1[a]) r[a]
  top = -J_tw/M_tot + r x bot
"""

import os
import sys

for _p in ("/opt/trn_rl_repo", "/root/.axon_site/_ro/trn_rl_repo"):
    if os.path.isdir(_p) and _p not in sys.path:
        sys.path.append(_p)

import numpy as np
import ml_dtypes

import concourse.bass as bass
import concourse.tile as tile
from concourse import bacc, mybir
from concourse.bass_utils import run_bass_kernel_spmd

# ----------------------------------------------------------------- constants
N_SAMPLES, N_HORIZON = 2048, 128
N_CORES = 8
P = 128
F = 64
SPC = N_SAMPLES // N_CORES * N_HORIZON  # 32768
NBLK = SPC // (P * F)  # 4

BASE_MASS, EEF_MASS = 100000.0, 243.66
MASS = np.array([105.98, 105.98, 314.98, 279.2, 105.98, 105.98, 243.66], np.float32)
DIAGS = np.array(
    [
        [12.19, 12.19, 3.061],
        [12.19, 12.19, 3.061],
        [15.41, 2094.71, 2103.19],
        [9.522, 1966.28, 1966.28],
        [8.305, 3.061, 8.0386],
        [12.13, 12.13, 3.061],
        [9.336, 44.41, 44.41],
    ],
    np.float32,
)
I0DIAG = np.array([69585.02, 69585.02, 66666.664], np.float32)

M_MAN = float(MASS.sum())
M_TOT = M_MAN + BASE_MASS + EEF_MASS
K = BASE_MASS + EEF_MASS
BETA = 6.65 * (243.66 / (100000.0 + 243.66))
DCUM = np.stack([DIAGS[j:].sum(0) for j in range(7)], axis=1)  # [a][j]
C1 = (DIAGS.sum(0) + I0DIAG).astype(np.float64)  # [a]

BF = mybir.dt.float16
NPBF = np.float16
SC = 64.0
ADD = mybir.AluOpType.add
MUL = mybir.AluOpType.mult

NCST = 42  # massc 21 | dcum 21


def _const_array() -> np.ndarray:
    row = np.concatenate(
        [
            np.tile(MASS / SC, 3),  # massc[a*7+i] = m_i / SC
            (DCUM / SC).reshape(21),  # dcum[a*7+j] / SC
        ]
    ).astype(NPBF)
    return np.ascontiguousarray(
        np.broadcast_to(row[None, :, None], (P, NCST, F))
    ).reshape(P, NCST * F)


def build_nc():
    nc = bacc.Bacc("TRN2")

    _nb = nc.alloc_sbuf_tensor("const-float32-negbeta", [128, 1], mybir.dt.float32)
    nc.gpsimd.memset(_nb.ap(), -BETA)
    nc.const_aps.aps[(mybir.dt.float32, -BETA)] = _nb.ap()
    nc.all_engine_barrier()

    x_in = nc.dram_tensor("x", [NBLK, P, 63 * F], BF, kind="ExternalInput")
    cst_in = nc.dram_tensor("cst", [P, NCST * F], BF, kind="ExternalInput")
    out_d = nc.dram_tensor("out", [NBLK, P, 42 * F], BF, kind="ExternalOutput")

    V = nc.vector
    G_ = nc.gpsimd

    def emul(E, out, a, b):
        E.scalar_tensor_tensor(out, a, 1.0, b, MUL, MUL)

    def eadd(E, out, a, b):
        E.scalar_tensor_tensor(out, a, 1.0, b, MUL, ADD)

    def esub(E, out, a, b):  # out = a - b
        E.scalar_tensor_tensor(out, b, -1.0, a, MUL, ADD)

    def efma(E, out, a, s, b):  # out = s*a + b
        E.scalar_tensor_tensor(out, a, s, b, MUL, ADD)

    with tile.TileContext(nc) as tc:
        with (
            tc.tile_pool(name="cstp", bufs=1) as cstp,
            tc.tile_pool(name="io", bufs=2) as io,
            tc.tile_pool(name="wk", bufs=2) as wk,
        ):
            cst = cstp.tile([P, NCST * F], BF, tag="cst")
            nc.scalar.dma_start(cst[:], cst_in[:])
            cv = cst[:].rearrange("p (c f) -> p c f", c=NCST, f=F)
            masscv = cv[:, 0:21, :].rearrange("p (a i) f -> p a i f", a=3, i=7)
            dcum3 = cst[:, 21 * F : 42 * F].rearrange(
                "p (a x) -> p a x", a=3, x=7 * F
            )

            def r2(t, n):  # [P, n, F]
                return t[:].rearrange("p (c f) -> p c f", c=n, f=F)

            def r3(t, a, i):  # [P, a, i, F]
                return t[:].rearrange("p (a i f) -> p a i f", a=a, i=i, f=F)

            def bj(v):  # [P,F] -> [P,7,F] broadcast over j (outermost)
                return v.unsqueeze(1).broadcast_to([P, 7, F])

            def front(b):
                st = {}
                xt = io.tile([P, 63 * F], BF, tag="xt")
                nc.sync.dma_start(xt[:], x_in[b])
                xv = r3(xt, 9, 7)
                Cv, Ppv, Jv = xv[:, 0:3], xv[:, 3:6], xv[:, 6:9]
                st["xv"], st["Jv"] = xv, Jv

                # DVE: rp, mc ; Pool: rt tree
                rp = wk.tile([P, 21 * F], BF, tag="rp")
                rpv = r3(rp, 3, 7)
                V.tensor_sub(rpv, Cv, Ppv)
                mc = wk.tile([P, 21 * F], BF, tag="mc")
                mcv = r3(mc, 3, 7)
                V.tensor_mul(mcv, masscv, Cv)
                y9 = wk.tile([P, 9 * F], BF, tag="y9")
                y9v = r3(y9, 3, 3)
                G_.tensor_add(y9v, mcv[:, :, 0:3, :], mcv[:, :, 3:6, :])
                rt = wk.tile([P, 3 * F], BF, tag="rt")
                rtv = r2(rt, 3)
                G_.tensor_add(rtv, y9v[:, :, 0, :], y9v[:, :, 1, :])
                G_.tensor_add(rtv, rtv, y9v[:, :, 2, :])
                G_.tensor_add(rtv, rtv, mcv[:, :, 6, :])
                rt_b4 = rtv.unsqueeze(2).broadcast_to([P, 3, 7, F])
                st["rtv"] = rtv

                # ACT: rs ; Pool: q ; ACT: g, w
                rs = wk.tile([P, 3 * F], BF, tag="rs")
                rsv = r2(rs, 3)
                nc.scalar.mul(rsv[:, 0:2, :], rtv[:, 0:2, :], SC / M_TOT)
                nc.scalar.activation(
                    rsv[:, 2, :],
                    rtv[:, 2, :],
                    mybir.ActivationFunctionType.Identity,
                    bias=-BETA,
                    scale=SC / M_TOT,
                )
                st["rsv"] = rsv
                q3 = wk.tile([P, 3 * F], BF, tag="q3")
                q3v = r2(q3, 3)
                G_.tensor_mul(q3v, rsv, rsv)
                q = wk.tile([P, F], BF, tag="q")
                qv = q[:]
                G_.tensor_add(qv, q3v[:, 0, :], q3v[:, 1, :])
                G_.tensor_add(qv, qv, q3v[:, 2, :])
                g = wk.tile([P, 3 * F], BF, tag="g")
                gv = r2(g, 3)
                w = wk.tile([P, 3 * F], BF, tag="w")
                wv = r2(w, 3)
                for a in range(3):
                    kc = float(K / C1[a])
                    nc.scalar.activation(
                        gv[:, a, :],
                        qv,
                        mybir.ActivationFunctionType.Identity,
                        bias=1.0,
                        scale=kc,
                    )
                    nc.scalar.mul(wv[:, a, :], rsv[:, a, :], kc)
                st["gv"], st["wv"] = gv, wv

                # DVE: scaled copies ; Pool: rj
                rtm = wk.tile([P, 3 * F], BF, tag="rtm")
                rtmv = r2(rtm, 3)
                V.tensor_scalar_mul(rtmv, rtv, -SC / M_TOT)
                jm = wk.tile([P, 21 * F], BF, tag="jm")
                jmv = r3(jm, 3, 7)
                V.tensor_scalar_mul(
                    r2(jm, 21),
                    xv[:, 6:9].rearrange("p a i f -> p (a i) f"),
                    -SC / M_TOT,
                )
                rjp = wk.tile([P, 21 * F], BF, tag="rjp")
                rjpv = r3(rjp, 3, 7)
                G_.tensor_mul(rjpv, rt_b4, Jv)
                rj = wk.tile([P, 7 * F], BF, tag="rj")
                rjv = r2(rj, 7)
                G_.tensor_add(rjv, rjpv[:, 0], rjpv[:, 1])
                G_.tensor_add(rjv, rjv, rjpv[:, 2])
                rjm = wk.tile([P, 7 * F], BF, tag="rjm")
                rjmv = r2(rjm, 7)
                V.tensor_scalar_mul(rjmv, rjv, SC / M_TOT)

                # DVE: U, suffix, trg
                ut = wk.tile([P, 84 * F], BF, tag="ut")
                Uv = ut[:].rearrange(
                    "p (a d i f) -> p a d i f", a=3, d=4, i=7, f=F
                )
                rp_b = rpv.unsqueeze(2).broadcast_to([P, 3, 3, 7, F])
                mc_b = mcv.unsqueeze(1).broadcast_to([P, 3, 3, 7, F])
                V.tensor_mul(Uv[:, :, 0:3], rp_b, mc_b)
                V.tensor_mul(Uv[:, :, 3], rpv, masscv)
                for j in range(5, -1, -1):
                    V.tensor_add(
                        Uv[:, :, :, j, :], Uv[:, :, :, j, :], Uv[:, :, :, j + 1, :]
                    )
                gd = Uv[:, :, 0:3]  # [P,3(a),3(d),7,F]
                rsuf = Uv[:, :, 3]  # [P,3,7,F]
                st["gd"], st["rsuf"] = gd, rsuf

                trg = wk.tile([P, 7 * F], BF, tag="trg")
                trgv = r2(trg, 7)
                V.tensor_add(trgv, gd[:, 0, 0], gd[:, 1, 1])
                V.tensor_add(trgv, trgv, gd[:, 2, 2])
                st["trgv"] = trgv

                # Pool: rrp', vr', ja/jb/jtw'
                rtm_b4 = rtmv.unsqueeze(2).broadcast_to([P, 3, 7, F])
                rrp = wk.tile([P, 21 * F], BF, tag="rrp")
                rrpv = r3(rrp, 3, 7)
                G_.tensor_mul(rrpv, rtm_b4, rsuf)
                st["rrpv"] = rrpv
                vr = wk.tile([P, 21 * F], BF, tag="vr")
                vr3 = vr[:].rearrange("p (a x) -> p a x", a=3, x=7 * F)
                rjm_b = rjm[:].unsqueeze(1).broadcast_to([P, 3, 7 * F])
                rsuf3 = rsuf.rearrange("p a i f -> p a (i f)")
                G_.tensor_mul(vr3, rjm_b, rsuf3)
                st["vr"] = vr
                ja = wk.tile([P, 21 * F], BF, tag="ja")
                jav = r3(ja, 3, 7)
                jb = wk.tile([P, 21 * F], BF, tag="jb")
                jbv = r3(jb, 3, 7)
                for a in range(3):
                    a1_, a2_ = (a + 1) % 3, (a + 2) % 3
                    G_.tensor_mul(jav[:, a], jmv[:, a1_], rsuf[:, a2_])
                    G_.tensor_mul(jbv[:, a], jmv[:, a2_], rsuf[:, a1_])
                G_.tensor_sub(r2(ja, 21), r2(ja, 21), r2(jb, 21))  # jtw'
                st["ja"] = ja
                return st

            def back(st, b):
                xv, Jv = st["xv"], st["Jv"]
                gd, rsuf, trgv = st["gd"], st["rsuf"], st["trgv"]
                rsv, gv, wv = st["rsv"], st["gv"], st["wv"]

                # DVE: T, rr', u, a1, hth
                tp = wk.tile([P, 63 * F], BF, tag="tp")
                tpv = tp[:].rearrange(
                    "p (a d j f) -> p a d j f", a=3, d=3, j=7, f=F
                )
                J_b = (
                    Jv.rearrange("p d j f -> p (d j) f")
                    .unsqueeze(1)
                    .broadcast_to([P, 3, 21, F])
                )
                V.tensor_mul(
                    tpv.rearrange("p a d j f -> p a (d j) f"),
                    gd.rearrange("p a d j f -> p a (d j) f"),
                    J_b,
                )
                tt = wk.tile([P, 21 * F], BF, tag="tt")
                ttv = r3(tt, 3, 7)
                V.tensor_add(ttv, tpv[:, :, 0], tpv[:, :, 1])
                V.tensor_add(ttv, ttv, tpv[:, :, 2])

                rrpv = st["rrpv"]
                rr = wk.tile([P, 7 * F], BF, tag="rr")
                rrv = r2(rr, 7)
                V.tensor_add(rrv, rrpv[:, 0], rrpv[:, 1])
                V.tensor_add(rrv, rrv, rrpv[:, 2])
                u7 = wk.tile([P, 7 * F], BF, tag="u7")
                u7v = r2(u7, 7)
                V.tensor_add(u7v, trgv, rrv)  # u = trg - rr/M
                a1 = wk.tile([P, 21 * F], BF, tag="a1")
                a13 = a1[:].rearrange("p (a x) -> p a x", a=3, x=7 * F)
                u_b = u7[:].unsqueeze(1).broadcast_to([P, 3, 7 * F])
                V.tensor_add(a13, dcum3, u_b)

                hth = wk.tile([P, 21 * F], BF, tag="hth")
                hthv = r3(hth, 3, 7)
                hthf = r2(hth, 21)
                V.tensor_mul(
                    hthf, r2(a1, 21), xv[:, 6:9].rearrange("p a i f -> p (a i) f")
                )
                V.tensor_sub(hthf, hthf, r2(tt, 21))
                V.tensor_add(hthf, hthf, r2(st["vr"], 21))

                # DVE: Y, s, bot, top
                Y = wk.tile([P, 21 * F], BF, tag="Y")
                Yv = r3(Y, 3, 7)
                for a in range(3):
                    V.tensor_scalar_mul(
                        Yv[:, a].rearrange("p i f -> p (i f)"),
                        hthv[:, a].rearrange("p i f -> p (i f)"),
                        float(SC / C1[a]),
                    )
                sp = wk.tile([P, 21 * F], BF, tag="sp")
                spv = r3(sp, 3, 7)
                rs_b4 = rsv.unsqueeze(2).broadcast_to([P, 3, 7, F])
                V.tensor_mul(spv, rs_b4, Yv)
                s7 = wk.tile([P, 7 * F], BF, tag="s7")
                s7v = r2(s7, 7)
                V.tensor_add(s7v, spv[:, 0], spv[:, 1])
                V.tensor_add(s7v, s7v, spv[:, 2])

                outt = io.tile([P, 42 * F], BF, tag="outt")
                outv = r3(outt, 6, 7)

                gy = wk.tile([P, 21 * F], BF, tag="gy")
                gyv = r3(gy, 3, 7)
                t1 = wk.tile([P, 21 * F], BF, tag="t1")
                t1v = r3(t1, 3, 7)
                g_b4 = gv.unsqueeze(2).broadcast_to([P, 3, 7, F])
                w_b4 = wv.unsqueeze(2).broadcast_to([P, 3, 7, F])
                s_b4 = s7v.unsqueeze(1).broadcast_to([P, 3, 7, F])
                V.tensor_mul(gyv, g_b4, Yv)
                V.tensor_mul(t1v, w_b4, s_b4)
                V.tensor_sub(
                    outv[:, 3:6].rearrange("p a j f -> p (a j) f"),
                    r2(t1, 21),
                    r2(gy, 21),
                )  # bot

                ctb = wk.tile([P, 21 * F], BF, tag="ctb")
                ctbv = r3(ctb, 3, 7)
                ctc = wk.tile([P, 21 * F], BF, tag="ctc")
                ctcv = r3(ctc, 3, 7)
                for a in range(3):
                    a1_, a2_ = (a + 1) % 3, (a + 2) % 3
                    V.tensor_mul(ctbv[:, a], bj(rsv[:, a1_, :]), outv[:, 3 + a2_])
                    V.tensor_mul(ctcv[:, a], bj(rsv[:, a2_, :]), outv[:, 3 + a1_])
                V.tensor_sub(r2(ctb, 21), r2(ctb, 21), r2(ctc, 21))
                V.tensor_add(
                    outv[:, 0:3].rearrange("p a j f -> p (a j) f"),
                    r2(st["ja"], 21),
                    r2(ctb, 21),
                )  # top = jtw' + r x bot

                nc.scalar.dma_start(out_d[b], outt[:])

            st_prev = None
            for b in range(NBLK):
                st = front(b)
                if st_prev is not None:
                    back(st_prev, b - 1)
                st_prev = st
            back(st_prev, NBLK - 1)

    nc.compile()
    return nc


_NC_CACHE = None


def _get_nc():
    global _NC_CACHE
    if _NC_CACHE is None:
        _NC_CACHE = build_nc()
    return _NC_CACHE


def _shard_inputs(com_list, link_pose_list, jacobian):
    S = N_SAMPLES * N_HORIZON
    com = np.asarray(com_list, np.float32).reshape(S, 21)
    pos = np.ascontiguousarray(
        np.asarray(link_pose_list, np.float32).reshape(S, 4, 4, 9)[:, 0:3, 3, 0:7]
    ).reshape(S, 21)
    j3 = np.ascontiguousarray(
        np.asarray(jacobian, np.float32).reshape(S, 6, 7)[:, 0:3, :]
    ).reshape(S, 21)
    x = np.concatenate([com, pos, j3], axis=1).astype(NPBF)  # (S, 63)
    x = np.ascontiguousarray(
        x.reshape(N_CORES, NBLK, P, F, 63).transpose(0, 1, 2, 4, 3)
    )  # (cores, NBLK, P, 63, F)
    cst = _const_array()
    return [
        {"x": x[c].reshape(NBLK, P, 63 * F), "cst": cst} for c in range(N_CORES)
    ]


def _gather(results):
    outs = np.stack([r["out"] for r in results])  # (8, NBLK, P, 42F) bf16
    o = outs.reshape(N_CORES, NBLK, P, 42, F).transpose(0, 1, 2, 4, 3)
    return np.ascontiguousarray(o).astype(np.float32).reshape(
        N_SAMPLES, N_HORIZON, 6, 7
    )


def run(com_list, link_pose_list, jacobian, trace=False):
    nc = _get_nc()
    in_maps = _shard_inputs(com_list, link_pose_list, jacobian)
    res = run_bass_kernel_spmd(nc, in_maps, list(range(N_CORES)), trace=trace)
    return _gather(res.results), res


def kernel(com_list, link_pose_list, jacobian):
    out, _ = run(com_list, link_pose_list, jacobian)
    return out


# revision 13
# speedup vs baseline: 1.0415x; 1.0415x over previous
"""Trainium2 Bass kernel for nn_CanadarmJacob (centroidal-dynamics jacobian).

Data-parallel over 8 NeuronCores; per core 32768 flat samples split into
NBLK=4 blocks of [P=128 partitions, F=64 free].  All per-sample quantities
live channel-major ([P, ch*F]) so every vector-op operand has a unit-stride
F-sized last dim -> DVE 2-byte fast modes apply.  Whole pipeline is bf16
scalar_tensor_tensor / tensor_scalar (InstTensorScalarPtr: 4x on DVE,
0.60-eff on Pool); ops are shaped so every access pattern canonicalizes to
<= 3 dims (stt verifier limit).  Only the 63 input floats/sample the
reference actually reads are shipped (com 21, link positions 21, jacobian
rows 0:3), packed host-side into one fused bf16 tensor.

Math (same validated algebra as the fp32 baseline):
  RP = C - P ;  MC = m_i*C ;  U[a,dd,i] = RP[a]*(MC[dd] | m_i)
  G = suffix_j(U) -> G[a,d,j], R[a,j] ;  rt = sum_i MC
  r = rt/M_tot - (0,0,beta) ;  T[a,j] = sum_d G[a,d,j]*J[d,j]
  trG ; rr = sum_a rt[a]R[a,j] ; rj = sum_a rt[a]J[a,j]
  u = trG - rr/M_tot ;  H_th = (DCUM + u)*J - T + (rj_b*R)/M_tot
  J_tw = J_j x R_j
H_s = K r r^T + diag(C1 - K|r|^2) = D + P_m with D = diag(C1) constant and
|P_m|/|D| <= ~4e-3, so first-order Neumann:  H_s^-1 ~= D^-1 - D^-1 P_m D^-1
  Y = Hth/C1 ;  s = sum_a r[a]Y[a,:] ;  q = |r|^2
  bot = w[a]*s - g[a]*Y ,  g = 1 + (K/C1[a]) q ,  w = (K/C1[a]) r[a]
  top = -J_tw/M_tot + r x bot
"""

import os
import sys

for _p in ("/opt/trn_rl_repo", "/root/.axon_site/_ro/trn_rl_repo"):
    if os.path.isdir(_p) and _p not in sys.path:
        sys.path.append(_p)

import numpy as np
import ml_dtypes

import concourse.bass as bass
import concourse.tile as tile
from concourse import bacc, mybir
from concourse.bass_utils import run_bass_kernel_spmd

# ----------------------------------------------------------------- constants
N_SAMPLES, N_HORIZON = 2048, 128
N_CORES = 8
P = 128
F = 64
SPC = N_SAMPLES // N_CORES * N_HORIZON  # 32768
NBLK = SPC // (P * F)  # 4

BASE_MASS, EEF_MASS = 100000.0, 243.66
MASS = np.array([105.98, 105.98, 314.98, 279.2, 105.98, 105.98, 243.66], np.float32)
DIAGS = np.array(
    [
        [12.19, 12.19, 3.061],
        [12.19, 12.19, 3.061],
        [15.41, 2094.71, 2103.19],
        [9.522, 1966.28, 1966.28],
        [8.305, 3.061, 8.0386],
        [12.13, 12.13, 3.061],
        [9.336, 44.41, 44.41],
    ],
    np.float32,
)
I0DIAG = np.array([69585.02, 69585.02, 66666.664], np.float32)

M_MAN = float(MASS.sum())
M_TOT = M_MAN + BASE_MASS + EEF_MASS
K = BASE_MASS + EEF_MASS
BETA = 6.65 * (243.66 / (100000.0 + 243.66))
DCUM = np.stack([DIAGS[j:].sum(0) for j in range(7)], axis=1)  # [a][j]
C1 = (DIAGS.sum(0) + I0DIAG).astype(np.float64)  # [a]

BF = mybir.dt.float16
NPBF = np.float16
SC = 64.0
ADD = mybir.AluOpType.add
MUL = mybir.AluOpType.mult

NCST = 42  # massc 21 | dcum 21


def _const_array() -> np.ndarray:
    row = np.concatenate(
        [
            np.tile(MASS / SC, 3),  # massc[a*7+i] = m_i / SC
            (DCUM / SC).reshape(21),  # dcum[a*7+j] / SC
        ]
    ).astype(NPBF)
    return np.ascontiguousarray(
        np.broadcast_to(row[None, :, None], (P, NCST, F))
    ).reshape(P, NCST * F)


def build_nc():
    nc = bacc.Bacc("TRN2")

    _nb = nc.alloc_sbuf_tensor("const-float32-negbeta", [128, 1], mybir.dt.float32)
    nc.gpsimd.memset(_nb.ap(), -BETA)
    nc.const_aps.aps[(mybir.dt.float32, -BETA)] = _nb.ap()
    nc.all_engine_barrier()

    x_in = nc.dram_tensor("x", [NBLK, P, 63 * F], BF, kind="ExternalInput")
    cst_in = nc.dram_tensor("cst", [P, NCST * F], BF, kind="ExternalInput")
    out_d = nc.dram_tensor("out", [NBLK, P, 42 * F], BF, kind="ExternalOutput")

    V = nc.vector
    G_ = nc.gpsimd

    def emul(E, out, a, b):
        E.scalar_tensor_tensor(out, a, 1.0, b, MUL, MUL)

    def eadd(E, out, a, b):
        E.scalar_tensor_tensor(out, a, 1.0, b, MUL, ADD)

    def esub(E, out, a, b):  # out = a - b
        E.scalar_tensor_tensor(out, b, -1.0, a, MUL, ADD)

    def efma(E, out, a, s, b):  # out = s*a + b
        E.scalar_tensor_tensor(out, a, s, b, MUL, ADD)

    with tile.TileContext(nc) as tc:
        with (
            tc.tile_pool(name="cstp", bufs=1) as cstp,
            tc.tile_pool(name="io", bufs=2) as io,
            tc.tile_pool(name="wk", bufs=2) as wk,
        ):
            cst = cstp.tile([P, NCST * F], BF, tag="cst")
            nc.scalar.dma_start(cst[:], cst_in[:])
            cv = cst[:].rearrange("p (c f) -> p c f", c=NCST, f=F)
            masscv = cv[:, 0:21, :].rearrange("p (a i) f -> p a i f", a=3, i=7)
            dcum3 = cst[:, 21 * F : 42 * F].rearrange(
                "p (a x) -> p a x", a=3, x=7 * F
            )

            def r2(t, n):  # [P, n, F]
                return t[:].rearrange("p (c f) -> p c f", c=n, f=F)

            def r3(t, a, i):  # [P, a, i, F]
                return t[:].rearrange("p (a i f) -> p a i f", a=a, i=i, f=F)

            def bj(v):  # [P,F] -> [P,7,F] broadcast over j (outermost)
                return v.unsqueeze(1).broadcast_to([P, 7, F])

            def front(b):
                st = {}
                xt = io.tile([P, 63 * F], BF, tag="xt")
                nc.sync.dma_start(xt[:], x_in[b])
                xv = r3(xt, 9, 7)
                Cv, Ppv, Jv = xv[:, 0:3], xv[:, 3:6], xv[:, 6:9]
                st["xv"], st["Jv"] = xv, Jv

                # DVE: rp, mc ; Pool: rt tree
                rp = wk.tile([P, 21 * F], BF, tag="rp")
                rpv = r3(rp, 3, 7)
                V.tensor_sub(rpv, Cv, Ppv)
                mc = wk.tile([P, 21 * F], BF, tag="mc")
                mcv = r3(mc, 3, 7)
                V.tensor_mul(mcv, masscv, Cv)
                y9 = wk.tile([P, 9 * F], BF, tag="y9")
                y9v = r3(y9, 3, 3)
                V.tensor_add(y9v, mcv[:, :, 0:3, :], mcv[:, :, 3:6, :])
                rt = wk.tile([P, 3 * F], BF, tag="rt")
                rtv = r2(rt, 3)
                V.tensor_add(rtv, y9v[:, :, 0, :], y9v[:, :, 1, :])
                V.tensor_add(rtv, rtv, y9v[:, :, 2, :])
                V.tensor_add(rtv, rtv, mcv[:, :, 6, :])
                rt_b4 = rtv.unsqueeze(2).broadcast_to([P, 3, 7, F])
                st["rtv"] = rtv

                # ACT: rs ; Pool: q ; ACT: g, w
                rs = wk.tile([P, 3 * F], BF, tag="rs")
                rsv = r2(rs, 3)
                nc.scalar.mul(rsv[:, 0:2, :], rtv[:, 0:2, :], SC / M_TOT)
                nc.scalar.activation(
                    rsv[:, 2, :],
                    rtv[:, 2, :],
                    mybir.ActivationFunctionType.Identity,
                    bias=-BETA,
                    scale=SC / M_TOT,
                )
                st["rsv"] = rsv
                q3 = wk.tile([P, 3 * F], BF, tag="q3")
                q3v = r2(q3, 3)
                G_.tensor_mul(q3v, rsv, rsv)
                q = wk.tile([P, F], BF, tag="q")
                qv = q[:]
                G_.tensor_add(qv, q3v[:, 0, :], q3v[:, 1, :])
                G_.tensor_add(qv, qv, q3v[:, 2, :])
                g = wk.tile([P, 3 * F], BF, tag="g")
                gv = r2(g, 3)
                w = wk.tile([P, 3 * F], BF, tag="w")
                wv = r2(w, 3)
                for a in range(3):
                    kc = float(K / C1[a])
                    nc.scalar.activation(
                        gv[:, a, :],
                        qv,
                        mybir.ActivationFunctionType.Identity,
                        bias=1.0,
                        scale=kc,
                    )
                    nc.scalar.mul(wv[:, a, :], rsv[:, a, :], kc)
                st["gv"], st["wv"] = gv, wv

                # DVE: jm ; Pool: rj
                jm = wk.tile([P, 21 * F], BF, tag="jm")
                jmv = r3(jm, 3, 7)
                V.tensor_scalar_mul(
                    r2(jm, 21),
                    xv[:, 6:9].rearrange("p a i f -> p (a i) f"),
                    -SC / M_TOT,
                )
                rjp = wk.tile([P, 21 * F], BF, tag="rjp")
                rjpv = r3(rjp, 3, 7)
                G_.tensor_mul(rjpv, rt_b4, Jv)
                rj = wk.tile([P, 7 * F], BF, tag="rj")
                rjv = r2(rj, 7)
                G_.tensor_add(rjv, rjpv[:, 0], rjpv[:, 1])
                G_.tensor_add(rjv, rjv, rjpv[:, 2])

                # DVE: U, suffix, trg
                ut = wk.tile([P, 84 * F], BF, tag="ut")
                Uv = ut[:].rearrange(
                    "p (a d i f) -> p a d i f", a=3, d=4, i=7, f=F
                )
                rp_b = rpv.unsqueeze(2).broadcast_to([P, 3, 3, 7, F])
                mc_b = mcv.unsqueeze(1).broadcast_to([P, 3, 3, 7, F])
                V.tensor_mul(Uv[:, :, 0:3], rp_b, mc_b)
                V.tensor_mul(Uv[:, :, 3], rpv, masscv)
                for j in range(5, -1, -1):
                    V.tensor_add(
                        Uv[:, :, :, j, :], Uv[:, :, :, j, :], Uv[:, :, :, j + 1, :]
                    )
                gd = Uv[:, :, 0:3]  # [P,3(a),3(d),7,F]
                rsuf = Uv[:, :, 3]  # [P,3,7,F]
                st["gd"], st["rsuf"] = gd, rsuf

                rtm = wk.tile([P, 3 * F], BF, tag="rtm")
                rtmv = r2(rtm, 3)
                V.tensor_scalar_mul(rtmv, rtv, -SC / M_TOT)
                rjm = wk.tile([P, 7 * F], BF, tag="rjm")
                rjmv = r2(rjm, 7)
                V.tensor_scalar_mul(rjmv, rjv, SC / M_TOT)

                trg = wk.tile([P, 7 * F], BF, tag="trg")
                trgv = r2(trg, 7)
                G_.tensor_add(trgv, gd[:, 0, 0], gd[:, 1, 1])
                G_.tensor_add(trgv, trgv, gd[:, 2, 2])
                st["trgv"] = trgv

                # Pool: rrp', vr', ja/jb/jtw'
                rtm_b4 = rtmv.unsqueeze(2).broadcast_to([P, 3, 7, F])
                rrp = wk.tile([P, 21 * F], BF, tag="rrp")
                rrpv = r3(rrp, 3, 7)
                G_.tensor_mul(rrpv, rtm_b4, rsuf)
                st["rrpv"] = rrpv
                vr = wk.tile([P, 21 * F], BF, tag="vr")
                vr3 = vr[:].rearrange("p (a x) -> p a x", a=3, x=7 * F)
                rjm_b = rjm[:].unsqueeze(1).broadcast_to([P, 3, 7 * F])
                rsuf3 = rsuf.rearrange("p a i f -> p a (i f)")
                G_.tensor_mul(vr3, rjm_b, rsuf3)
                st["vr"] = vr
                ja = wk.tile([P, 21 * F], BF, tag="ja")
                jav = r3(ja, 3, 7)
                jb = wk.tile([P, 21 * F], BF, tag="jb")
                jbv = r3(jb, 3, 7)
                for a in range(3):
                    a1_, a2_ = (a + 1) % 3, (a + 2) % 3
                    G_.tensor_mul(jav[:, a], jmv[:, a1_], rsuf[:, a2_])
                    G_.tensor_mul(jbv[:, a], jmv[:, a2_], rsuf[:, a1_])
                G_.tensor_sub(r2(ja, 21), r2(ja, 21), r2(jb, 21))  # jtw'
                st["ja"] = ja
                return st

            def back(st, b):
                xv, Jv = st["xv"], st["Jv"]
                gd, rsuf, trgv = st["gd"], st["rsuf"], st["trgv"]
                rsv, gv, wv = st["rsv"], st["gv"], st["wv"]

                # DVE: T, rr', u, a1, hth
                tp = wk.tile([P, 63 * F], BF, tag="tp")
                tpv = tp[:].rearrange(
                    "p (a d j f) -> p a d j f", a=3, d=3, j=7, f=F
                )
                J_b = (
                    Jv.rearrange("p d j f -> p (d j) f")
                    .unsqueeze(1)
                    .broadcast_to([P, 3, 21, F])
                )
                V.tensor_mul(
                    tpv.rearrange("p a d j f -> p a (d j) f"),
                    gd.rearrange("p a d j f -> p a (d j) f"),
                    J_b,
                )
                tt = wk.tile([P, 21 * F], BF, tag="tt")
                ttv = r3(tt, 3, 7)
                V.tensor_add(ttv, tpv[:, :, 0], tpv[:, :, 1])
                V.tensor_add(ttv, ttv, tpv[:, :, 2])

                rrpv = st["rrpv"]
                rr = wk.tile([P, 7 * F], BF, tag="rr")
                rrv = r2(rr, 7)
                V.tensor_add(rrv, rrpv[:, 0], rrpv[:, 1])
                V.tensor_add(rrv, rrv, rrpv[:, 2])
                u7 = wk.tile([P, 7 * F], BF, tag="u7")
                u7v = r2(u7, 7)
                V.tensor_add(u7v, trgv, rrv)  # u = trg - rr/M
                a1 = wk.tile([P, 21 * F], BF, tag="a1")
                a13 = a1[:].rearrange("p (a x) -> p a x", a=3, x=7 * F)
                u_b = u7[:].unsqueeze(1).broadcast_to([P, 3, 7 * F])
                V.tensor_add(a13, dcum3, u_b)

                hth = wk.tile([P, 21 * F], BF, tag="hth")
                hthv = r3(hth, 3, 7)
                hthf = r2(hth, 21)
                V.tensor_mul(
                    hthf, r2(a1, 21), xv[:, 6:9].rearrange("p a i f -> p (a i) f")
                )
                V.tensor_sub(hthf, hthf, r2(tt, 21))
                V.tensor_add(hthf, hthf, r2(st["vr"], 21))

                # DVE: Y, s, bot, top
                Y = wk.tile([P, 21 * F], BF, tag="Y")
                Yv = r3(Y, 3, 7)
                for a in range(3):
                    V.tensor_scalar_mul(
                        Yv[:, a].rearrange("p i f -> p (i f)"),
                        hthv[:, a].rearrange("p i f -> p (i f)"),
                        float(SC / C1[a]),
                    )
                sp = wk.tile([P, 21 * F], BF, tag="sp")
                spv = r3(sp, 3, 7)
                rs_b4 = rsv.unsqueeze(2).broadcast_to([P, 3, 7, F])
                V.tensor_mul(spv, rs_b4, Yv)
                s7 = wk.tile([P, 7 * F], BF, tag="s7")
                s7v = r2(s7, 7)
                V.tensor_add(s7v, spv[:, 0], spv[:, 1])
                V.tensor_add(s7v, s7v, spv[:, 2])

                outt = io.tile([P, 42 * F], BF, tag="outt")
                outv = r3(outt, 6, 7)

                gy = wk.tile([P, 21 * F], BF, tag="gy")
                gyv = r3(gy, 3, 7)
                t1 = wk.tile([P, 21 * F], BF, tag="t1")
                t1v = r3(t1, 3, 7)
                g_b4 = gv.unsqueeze(2).broadcast_to([P, 3, 7, F])
                w_b4 = wv.unsqueeze(2).broadcast_to([P, 3, 7, F])
                s_b4 = s7v.unsqueeze(1).broadcast_to([P, 3, 7, F])
                V.tensor_mul(gyv, g_b4, Yv)
                V.tensor_mul(t1v, w_b4, s_b4)
                V.tensor_sub(
                    outv[:, 3:6].rearrange("p a j f -> p (a j) f"),
                    r2(t1, 21),
                    r2(gy, 21),
                )  # bot

                ctb = wk.tile([P, 21 * F], BF, tag="ctb")
                ctbv = r3(ctb, 3, 7)
                ctc = wk.tile([P, 21 * F], BF, tag="ctc")
                ctcv = r3(ctc, 3, 7)
                for a in range(3):
                    a1_, a2_ = (a + 1) % 3, (a + 2) % 3
                    V.tensor_mul(ctbv[:, a], bj(rsv[:, a1_, :]), outv[:, 3 + a2_])
                    V.tensor_mul(ctcv[:, a], bj(rsv[:, a2_, :]), outv[:, 3 + a1_])
                V.tensor_sub(r2(ctb, 21), r2(ctb, 21), r2(ctc, 21))
                V.tensor_add(
                    outv[:, 0:3].rearrange("p a j f -> p (a j) f"),
                    r2(st["ja"], 21),
                    r2(ctb, 21),
                )  # top = jtw' + r x bot

                nc.scalar.dma_start(out_d[b], outt[:])

            st_prev = None
            for b in range(NBLK):
                st = front(b)
                if st_prev is not None:
                    back(st_prev, b - 1)
                st_prev = st
            back(st_prev, NBLK - 1)

    nc.compile()
    return nc


_NC_CACHE = None


def _get_nc():
    global _NC_CACHE
    if _NC_CACHE is None:
        _NC_CACHE = build_nc()
    return _NC_CACHE


def _shard_inputs(com_list, link_pose_list, jacobian):
    S = N_SAMPLES * N_HORIZON
    com = np.asarray(com_list, np.float32).reshape(S, 21)
    pos = np.ascontiguousarray(
        np.asarray(link_pose_list, np.float32).reshape(S, 4, 4, 9)[:, 0:3, 3, 0:7]
    ).reshape(S, 21)
    j3 = np.ascontiguousarray(
        np.asarray(jacobian, np.float32).reshape(S, 6, 7)[:, 0:3, :]
    ).reshape(S, 21)
    x = np.concatenate([com, pos, j3], axis=1).astype(NPBF)  # (S, 63)
    x = np.ascontiguousarray(
        x.reshape(N_CORES, NBLK, P, F, 63).transpose(0, 1, 2, 4, 3)
    )  # (cores, NBLK, P, 63, F)
    cst = _const_array()
    return [
        {"x": x[c].reshape(NBLK, P, 63 * F), "cst": cst} for c in range(N_CORES)
    ]


def _gather(results):
    outs = np.stack([r["out"] for r in results])  # (8, NBLK, P, 42F) bf16
    o = outs.reshape(N_CORES, NBLK, P, 42, F).transpose(0, 1, 2, 4, 3)
    return np.ascontiguousarray(o).astype(np.float32).reshape(
        N_SAMPLES, N_HORIZON, 6, 7
    )


def run(com_list, link_pose_list, jacobian, trace=False):
    nc = _get_nc()
    in_maps = _shard_inputs(com_list, link_pose_list, jacobian)
    res = run_bass_kernel_spmd(nc, in_maps, list(range(N_CORES)), trace=trace)
    return _gather(res.results), res


def kernel(com_list, link_pose_list, jacobian):
    out, _ = run(com_list, link_pose_list, jacobian)
    return out


# revision 14
# speedup vs baseline: 1.1717x; 1.1250x over previous
"""Trainium2 Bass kernel for nn_CanadarmJacob (centroidal-dynamics jacobian).

Data-parallel over 8 NeuronCores; per core 32768 flat samples split into
NBLK=4 blocks of [P=128 partitions, F=64 free].  All per-sample quantities
live channel-major ([P, ch*F]) so every vector-op operand has a unit-stride
F-sized last dim -> DVE 2-byte fast modes apply.  Whole pipeline is bf16
scalar_tensor_tensor / tensor_scalar (InstTensorScalarPtr: 4x on DVE,
0.60-eff on Pool); ops are shaped so every access pattern canonicalizes to
<= 3 dims (stt verifier limit).  Only the 63 input floats/sample the
reference actually reads are shipped (com 21, link positions 21, jacobian
rows 0:3), packed host-side into one fused bf16 tensor.

Math (same validated algebra as the fp32 baseline):
  RP = C - P ;  MC = m_i*C ;  U[a,dd,i] = RP[a]*(MC[dd] | m_i)
  G = suffix_j(U) -> G[a,d,j], R[a,j] ;  rt = sum_i MC
  r = rt/M_tot - (0,0,beta) ;  T[a,j] = sum_d G[a,d,j]*J[d,j]
  trG ; rr = sum_a rt[a]R[a,j] ; rj = sum_a rt[a]J[a,j]
  u = trG - rr/M_tot ;  H_th = (DCUM + u)*J - T + (rj_b*R)/M_tot
  J_tw = J_j x R_j
H_s = K r r^T + diag(C1 - K|r|^2) = D + P_m with D = diag(C1) constant and
|P_m|/|D| <= ~4e-3, so first-order Neumann:  H_s^-1 ~= D^-1 - D^-1 P_m D^-1
  Y = Hth/C1 ;  s = sum_a r[a]Y[a,:] ;  q = |r|^2
  bot = w[a]*s - g[a]*Y ,  g = 1 + (K/C1[a]) q ,  w = (K/C1[a]) r[a]
  top = -J_tw/M_tot + r x bot
"""

import os
import sys

for _p in ("/opt/trn_rl_repo", "/root/.axon_site/_ro/trn_rl_repo"):
    if os.path.isdir(_p) and _p not in sys.path:
        sys.path.append(_p)

import numpy as np
import ml_dtypes

import concourse.bass as bass
import concourse.tile as tile
from concourse import bacc, mybir
from concourse.bass_utils import run_bass_kernel_spmd

# ----------------------------------------------------------------- constants
N_SAMPLES, N_HORIZON = 2048, 128
N_CORES = 8
P = 128
F = 64
SPC = N_SAMPLES // N_CORES * N_HORIZON  # 32768
NBLK = SPC // (P * F)  # 4

BASE_MASS, EEF_MASS = 100000.0, 243.66
MASS = np.array([105.98, 105.98, 314.98, 279.2, 105.98, 105.98, 243.66], np.float32)
DIAGS = np.array(
    [
        [12.19, 12.19, 3.061],
        [12.19, 12.19, 3.061],
        [15.41, 2094.71, 2103.19],
        [9.522, 1966.28, 1966.28],
        [8.305, 3.061, 8.0386],
        [12.13, 12.13, 3.061],
        [9.336, 44.41, 44.41],
    ],
    np.float32,
)
I0DIAG = np.array([69585.02, 69585.02, 66666.664], np.float32)

M_MAN = float(MASS.sum())
M_TOT = M_MAN + BASE_MASS + EEF_MASS
K = BASE_MASS + EEF_MASS
BETA = 6.65 * (243.66 / (100000.0 + 243.66))
DCUM = np.stack([DIAGS[j:].sum(0) for j in range(7)], axis=1)  # [a][j]
C1 = (DIAGS.sum(0) + I0DIAG).astype(np.float64)  # [a]

BF = mybir.dt.float16
NPBF = np.float16
SC = 64.0
ADD = mybir.AluOpType.add
MUL = mybir.AluOpType.mult

NCST = 42  # massc 21 | dcum 21


def _const_array() -> np.ndarray:
    row = np.concatenate(
        [
            np.tile(MASS / SC, 3),  # massc[a*7+i] = m_i / SC
            (DCUM / SC).reshape(21),  # dcum[a*7+j] / SC
        ]
    ).astype(NPBF)
    return np.ascontiguousarray(
        np.broadcast_to(row[None, :, None], (P, NCST, F))
    ).reshape(P, NCST * F)


def build_nc():
    nc = bacc.Bacc("TRN2")

    _nb = nc.alloc_sbuf_tensor("const-float32-negbeta", [128, 1], mybir.dt.float32)
    nc.gpsimd.memset(_nb.ap(), -BETA)
    nc.const_aps.aps[(mybir.dt.float32, -BETA)] = _nb.ap()
    nc.all_engine_barrier()

    x_in = nc.dram_tensor("x", [NBLK, P, 63 * F], BF, kind="ExternalInput")
    cst_in = nc.dram_tensor("cst", [P, NCST * F], BF, kind="ExternalInput")
    out_d = nc.dram_tensor("out", [NBLK, P, 42 * F], BF, kind="ExternalOutput")

    V = nc.vector
    G_ = nc.gpsimd

    def emul(E, out, a, b):
        E.scalar_tensor_tensor(out, a, 1.0, b, MUL, MUL)

    def eadd(E, out, a, b):
        E.scalar_tensor_tensor(out, a, 1.0, b, MUL, ADD)

    def esub(E, out, a, b):  # out = a - b
        E.scalar_tensor_tensor(out, b, -1.0, a, MUL, ADD)

    def efma(E, out, a, s, b):  # out = s*a + b
        E.scalar_tensor_tensor(out, a, s, b, MUL, ADD)

    with tile.TileContext(nc) as tc:
        with (
            tc.tile_pool(name="cstp", bufs=1) as cstp,
            tc.tile_pool(name="io", bufs=2) as io,
            tc.tile_pool(name="wk", bufs=2) as wk,
        ):
            cst = cstp.tile([P, NCST * F], BF, tag="cst")
            nc.scalar.dma_start(cst[:], cst_in[:])
            cv = cst[:].rearrange("p (c f) -> p c f", c=NCST, f=F)
            masscv = cv[:, 0:21, :].rearrange("p (a i) f -> p a i f", a=3, i=7)
            dcum3 = cst[:, 21 * F : 42 * F].rearrange(
                "p (a x) -> p a x", a=3, x=7 * F
            )

            def r2(t, n):  # [P, n, F]
                return t[:].rearrange("p (c f) -> p c f", c=n, f=F)

            def r3(t, a, i):  # [P, a, i, F]
                return t[:].rearrange("p (a i f) -> p a i f", a=a, i=i, f=F)

            def bj(v):  # [P,F] -> [P,7,F] broadcast over j (outermost)
                return v.unsqueeze(1).broadcast_to([P, 7, F])

            def front(b):
                st = {}
                xt = io.tile([P, 63 * F], BF, tag="xt")
                nc.sync.dma_start(xt[:, 0 : 42 * F], x_in[b, :, 0 : 42 * F])
                nc.sync.dma_start(xt[:, 42 * F :], x_in[b, :, 42 * F :])
                xv = r3(xt, 9, 7)
                Cv, Ppv, Jv = xv[:, 0:3], xv[:, 3:6], xv[:, 6:9]
                st["xv"], st["Jv"] = xv, Jv

                # DVE: rp, mc ; Pool: rt tree
                rp = wk.tile([P, 21 * F], BF, tag="rp")
                rpv = r3(rp, 3, 7)
                V.tensor_sub(rpv, Cv, Ppv)
                mc = wk.tile([P, 21 * F], BF, tag="mc")
                mcv = r3(mc, 3, 7)
                V.tensor_mul(mcv, masscv, Cv)
                y9 = wk.tile([P, 9 * F], BF, tag="y9")
                y9v = r3(y9, 3, 3)
                V.tensor_add(y9v, mcv[:, :, 0:3, :], mcv[:, :, 3:6, :])
                rt = wk.tile([P, 3 * F], BF, tag="rt")
                rtv = r2(rt, 3)
                V.tensor_add(rtv, y9v[:, :, 0, :], y9v[:, :, 1, :])
                V.tensor_add(rtv, rtv, y9v[:, :, 2, :])
                V.tensor_add(rtv, rtv, mcv[:, :, 6, :])
                st["rtv"] = rtv
                rtm = wk.tile([P, 3 * F], BF, tag="rtm")
                rtmv = r2(rtm, 3)
                V.tensor_scalar_mul(rtmv, rtv, -SC / M_TOT)
                rtm_b4 = rtmv.unsqueeze(2).broadcast_to([P, 3, 7, F])
                st["rtm_b4"] = rtm_b4

                # ACT: rs ; Pool: q ; ACT: g, w
                rs = wk.tile([P, 3 * F], BF, tag="rs")
                rsv = r2(rs, 3)
                nc.scalar.mul(rsv[:, 0:2, :], rtv[:, 0:2, :], SC / M_TOT)
                nc.scalar.activation(
                    rsv[:, 2, :],
                    rtv[:, 2, :],
                    mybir.ActivationFunctionType.Identity,
                    bias=-BETA,
                    scale=SC / M_TOT,
                )
                st["rsv"] = rsv
                q3 = wk.tile([P, 3 * F], BF, tag="q3")
                q3v = r2(q3, 3)
                G_.tensor_mul(q3v, rsv, rsv)
                q = wk.tile([P, F], BF, tag="q")
                qv = q[:]
                G_.tensor_add(qv, q3v[:, 0, :], q3v[:, 1, :])
                G_.tensor_add(qv, qv, q3v[:, 2, :])
                g = wk.tile([P, 3 * F], BF, tag="g")
                gv = r2(g, 3)
                w = wk.tile([P, 3 * F], BF, tag="w")
                wv = r2(w, 3)
                for a in range(3):
                    kc = float(K / C1[a])
                    nc.scalar.activation(
                        gv[:, a, :],
                        qv,
                        mybir.ActivationFunctionType.Identity,
                        bias=1.0,
                        scale=kc,
                    )
                    nc.scalar.mul(wv[:, a, :], rsv[:, a, :], kc)
                st["gv"], st["wv"] = gv, wv

                # DVE: jm ; Pool: rj
                jm = wk.tile([P, 21 * F], BF, tag="jm")
                jmv = r3(jm, 3, 7)
                V.tensor_scalar_mul(
                    r2(jm, 21),
                    xv[:, 6:9].rearrange("p a i f -> p (a i) f"),
                    -SC / M_TOT,
                )
                rjp = wk.tile([P, 21 * F], BF, tag="rjp")
                rjpv = r3(rjp, 3, 7)
                G_.tensor_mul(rjpv, rtm_b4, Jv)  # rjp' = -(rt/M SC)_b * J
                rj = wk.tile([P, 7 * F], BF, tag="rj")
                rjv = r2(rj, 7)
                G_.tensor_add(rjv, rjpv[:, 0], rjpv[:, 1])
                G_.tensor_add(rjv, rjv, rjpv[:, 2])
                st["rj"] = rj

                # DVE: U, suffix, trg
                ut = wk.tile([P, 84 * F], BF, tag="ut")
                Uv = ut[:].rearrange(
                    "p (a d i f) -> p a d i f", a=3, d=4, i=7, f=F
                )
                rp_b = rpv.unsqueeze(2).broadcast_to([P, 3, 3, 7, F])
                mc_b = mcv.unsqueeze(1).broadcast_to([P, 3, 3, 7, F])
                V.tensor_mul(Uv[:, :, 0:3], rp_b, mc_b)
                V.tensor_mul(Uv[:, :, 3], rpv, masscv)
                for j in range(5, -1, -1):
                    V.tensor_add(
                        Uv[:, :, :, j, :], Uv[:, :, :, j, :], Uv[:, :, :, j + 1, :]
                    )
                gd = Uv[:, :, 0:3]  # [P,3(a),3(d),7,F]
                rsuf = Uv[:, :, 3]  # [P,3,7,F]
                st["gd"], st["rsuf"] = gd, rsuf

                st["jmv"] = jmv
                return st

            def back(st, b):
                xv, Jv = st["xv"], st["Jv"]
                gd, rsuf = st["gd"], st["rsuf"]
                rsv, gv, wv = st["rsv"], st["gv"], st["wv"]
                rtm_b4, jmv = st["rtm_b4"], st["jmv"]

                # Pool phase B for this block (issued here so Pool A of the
                # next block runs first in Pool program order)
                trg = wk.tile([P, 7 * F], BF, tag="trg")
                trgv = r2(trg, 7)
                G_.tensor_add(trgv, gd[:, 0, 0], gd[:, 1, 1])
                G_.tensor_add(trgv, trgv, gd[:, 2, 2])
                rrp = wk.tile([P, 21 * F], BF, tag="rrp")
                rrpv = r3(rrp, 3, 7)
                G_.tensor_mul(rrpv, rtm_b4, rsuf)
                vr = wk.tile([P, 21 * F], BF, tag="vr")
                vr3 = vr[:].rearrange("p (a x) -> p a x", a=3, x=7 * F)
                rj_b = st["rj"][:].unsqueeze(1).broadcast_to([P, 3, 7 * F])
                rsuf3 = rsuf.rearrange("p a i f -> p a (i f)")
                G_.tensor_mul(vr3, rj_b, rsuf3)  # vr' = -vr*SC/M
                ja = wk.tile([P, 21 * F], BF, tag="ja")
                jav = r3(ja, 3, 7)
                jb = wk.tile([P, 21 * F], BF, tag="jb")
                jbv = r3(jb, 3, 7)
                for a in range(3):
                    a1_, a2_ = (a + 1) % 3, (a + 2) % 3
                    G_.tensor_mul(jav[:, a], jmv[:, a1_], rsuf[:, a2_])
                    G_.tensor_mul(jbv[:, a], jmv[:, a2_], rsuf[:, a1_])
                G_.tensor_sub(r2(ja, 21), r2(ja, 21), r2(jb, 21))  # jtw'

                # DVE: T, rr', u, a1, hth
                tp = wk.tile([P, 63 * F], BF, tag="tp")
                tpv = tp[:].rearrange(
                    "p (a d j f) -> p a d j f", a=3, d=3, j=7, f=F
                )
                J_b = (
                    Jv.rearrange("p d j f -> p (d j) f")
                    .unsqueeze(1)
                    .broadcast_to([P, 3, 21, F])
                )
                V.tensor_mul(
                    tpv.rearrange("p a d j f -> p a (d j) f"),
                    gd.rearrange("p a d j f -> p a (d j) f"),
                    J_b,
                )
                tt = wk.tile([P, 21 * F], BF, tag="tt")
                ttv = r3(tt, 3, 7)
                V.tensor_add(ttv, tpv[:, :, 0], tpv[:, :, 1])
                V.tensor_add(ttv, ttv, tpv[:, :, 2])

                rr = wk.tile([P, 7 * F], BF, tag="rr")
                rrv = r2(rr, 7)
                V.tensor_add(rrv, rrpv[:, 0], rrpv[:, 1])
                V.tensor_add(rrv, rrv, rrpv[:, 2])
                u7 = wk.tile([P, 7 * F], BF, tag="u7")
                u7v = r2(u7, 7)
                V.tensor_add(u7v, trgv, rrv)  # u = trg - rr/M
                a1 = wk.tile([P, 21 * F], BF, tag="a1")
                a13 = a1[:].rearrange("p (a x) -> p a x", a=3, x=7 * F)
                u_b = u7[:].unsqueeze(1).broadcast_to([P, 3, 7 * F])
                V.tensor_add(a13, dcum3, u_b)

                hth = wk.tile([P, 21 * F], BF, tag="hth")
                hthv = r3(hth, 3, 7)
                hthf = r2(hth, 21)
                V.tensor_mul(
                    hthf, r2(a1, 21), xv[:, 6:9].rearrange("p a i f -> p (a i) f")
                )
                V.tensor_sub(hthf, hthf, r2(tt, 21))
                V.tensor_sub(hthf, hthf, r2(vr, 21))

                # DVE: Y, s, bot, top
                Y = wk.tile([P, 21 * F], BF, tag="Y")
                Yv = r3(Y, 3, 7)
                for a in range(3):
                    V.tensor_scalar_mul(
                        Yv[:, a].rearrange("p i f -> p (i f)"),
                        hthv[:, a].rearrange("p i f -> p (i f)"),
                        float(SC / C1[a]),
                    )
                sp = wk.tile([P, 21 * F], BF, tag="sp")
                spv = r3(sp, 3, 7)
                rs_b4 = rsv.unsqueeze(2).broadcast_to([P, 3, 7, F])
                V.tensor_mul(spv, rs_b4, Yv)
                s7 = wk.tile([P, 7 * F], BF, tag="s7")
                s7v = r2(s7, 7)
                V.tensor_add(s7v, spv[:, 0], spv[:, 1])
                V.tensor_add(s7v, s7v, spv[:, 2])

                outt = io.tile([P, 42 * F], BF, tag="outt")
                outv = r3(outt, 6, 7)

                gy = wk.tile([P, 21 * F], BF, tag="gy")
                gyv = r3(gy, 3, 7)
                t1 = wk.tile([P, 21 * F], BF, tag="t1")
                t1v = r3(t1, 3, 7)
                g_b4 = gv.unsqueeze(2).broadcast_to([P, 3, 7, F])
                w_b4 = wv.unsqueeze(2).broadcast_to([P, 3, 7, F])
                s_b4 = s7v.unsqueeze(1).broadcast_to([P, 3, 7, F])
                V.tensor_mul(gyv, g_b4, Yv)
                V.tensor_mul(t1v, w_b4, s_b4)
                V.tensor_sub(
                    outv[:, 3:6].rearrange("p a j f -> p (a j) f"),
                    r2(t1, 21),
                    r2(gy, 21),
                )  # bot

                ctb = wk.tile([P, 21 * F], BF, tag="ctb")
                ctbv = r3(ctb, 3, 7)
                ctc = wk.tile([P, 21 * F], BF, tag="ctc")
                ctcv = r3(ctc, 3, 7)
                for a in range(3):
                    a1_, a2_ = (a + 1) % 3, (a + 2) % 3
                    V.tensor_mul(ctbv[:, a], bj(rsv[:, a1_, :]), outv[:, 3 + a2_])
                    V.tensor_mul(ctcv[:, a], bj(rsv[:, a2_, :]), outv[:, 3 + a1_])
                V.tensor_sub(r2(ctb, 21), r2(ctb, 21), r2(ctc, 21))
                V.tensor_add(
                    outv[:, 0:3].rearrange("p a j f -> p (a j) f"),
                    r2(ja, 21),
                    r2(ctb, 21),
                )  # top = jtw' + r x bot

                nc.scalar.dma_start(out_d[b], outt[:])

            st_prev = None
            for b in range(NBLK):
                st = front(b)
                if st_prev is not None:
                    back(st_prev, b - 1)
                st_prev = st
            back(st_prev, NBLK - 1)

    nc.compile()
    return nc


_NC_CACHE = None


def _get_nc():
    global _NC_CACHE
    if _NC_CACHE is None:
        _NC_CACHE = build_nc()
    return _NC_CACHE


def _shard_inputs(com_list, link_pose_list, jacobian):
    S = N_SAMPLES * N_HORIZON
    com = np.asarray(com_list, np.float32).reshape(S, 21)
    pos = np.ascontiguousarray(
        np.asarray(link_pose_list, np.float32).reshape(S, 4, 4, 9)[:, 0:3, 3, 0:7]
    ).reshape(S, 21)
    j3 = np.ascontiguousarray(
        np.asarray(jacobian, np.float32).reshape(S, 6, 7)[:, 0:3, :]
    ).reshape(S, 21)
    x = np.concatenate([com, pos, j3], axis=1).astype(NPBF)  # (S, 63)
    x = np.ascontiguousarray(
        x.reshape(N_CORES, NBLK, P, F, 63).transpose(0, 1, 2, 4, 3)
    )  # (cores, NBLK, P, 63, F)
    cst = _const_array()
    return [
        {"x": x[c].reshape(NBLK, P, 63 * F), "cst": cst} for c in range(N_CORES)
    ]


def _gather(results):
    outs = np.stack([r["out"] for r in results])  # (8, NBLK, P, 42F) bf16
    o = outs.reshape(N_CORES, NBLK, P, 42, F).transpose(0, 1, 2, 4, 3)
    return np.ascontiguousarray(o).astype(np.float32).reshape(
        N_SAMPLES, N_HORIZON, 6, 7
    )


def run(com_list, link_pose_list, jacobian, trace=False):
    nc = _get_nc()
    in_maps = _shard_inputs(com_list, link_pose_list, jacobian)
    res = run_bass_kernel_spmd(nc, in_maps, list(range(N_CORES)), trace=trace)
    return _gather(res.results), res


def kernel(com_list, link_pose_list, jacobian):
    out, _ = run(com_list, link_pose_list, jacobian)
    return out


# revision 15
# speedup vs baseline: 1.1971x; 1.0217x over previous
"""Trainium2 Bass kernel for nn_CanadarmJacob (centroidal-dynamics jacobian).

Data-parallel over 8 NeuronCores; per core 32768 flat samples split into
NBLK=4 blocks of [P=128 partitions, F=64 free].  All per-sample quantities
live channel-major ([P, ch*F]) so every vector-op operand has a unit-stride
F-sized last dim -> DVE 2-byte fast modes apply.  Whole pipeline is bf16
scalar_tensor_tensor / tensor_scalar (InstTensorScalarPtr: 4x on DVE,
0.60-eff on Pool); ops are shaped so every access pattern canonicalizes to
<= 3 dims (stt verifier limit).  Only the 63 input floats/sample the
reference actually reads are shipped (com 21, link positions 21, jacobian
rows 0:3), packed host-side into one fused bf16 tensor.

Math (same validated algebra as the fp32 baseline):
  RP = C - P ;  MC = m_i*C ;  U[a,dd,i] = RP[a]*(MC[dd] | m_i)
  G = suffix_j(U) -> G[a,d,j], R[a,j] ;  rt = sum_i MC
  r = rt/M_tot - (0,0,beta) ;  T[a,j] = sum_d G[a,d,j]*J[d,j]
  trG ; rr = sum_a rt[a]R[a,j] ; rj = sum_a rt[a]J[a,j]
  u = trG - rr/M_tot ;  H_th = (DCUM + u)*J - T + (rj_b*R)/M_tot
  J_tw = J_j x R_j
H_s = K r r^T + diag(C1 - K|r|^2) = D + P_m with D = diag(C1) constant and
|P_m|/|D| <= ~4e-3, so first-order Neumann:  H_s^-1 ~= D^-1 - D^-1 P_m D^-1
  Y = Hth/C1 ;  s = sum_a r[a]Y[a,:] ;  q = |r|^2
  bot = w[a]*s - g[a]*Y ,  g = 1 + (K/C1[a]) q ,  w = (K/C1[a]) r[a]
  top = -J_tw/M_tot + r x bot
"""

import os
import sys

for _p in ("/opt/trn_rl_repo", "/root/.axon_site/_ro/trn_rl_repo"):
    if os.path.isdir(_p) and _p not in sys.path:
        sys.path.append(_p)

import numpy as np
import ml_dtypes

import concourse.bass as bass
import concourse.tile as tile
from concourse import bacc, mybir
from concourse.bass_utils import run_bass_kernel_spmd

# ----------------------------------------------------------------- constants
N_SAMPLES, N_HORIZON = 2048, 128
N_CORES = 8
P = 128
F = 64
SPC = N_SAMPLES // N_CORES * N_HORIZON  # 32768
NBLK = SPC // (P * F)  # 4

BASE_MASS, EEF_MASS = 100000.0, 243.66
MASS = np.array([105.98, 105.98, 314.98, 279.2, 105.98, 105.98, 243.66], np.float32)
DIAGS = np.array(
    [
        [12.19, 12.19, 3.061],
        [12.19, 12.19, 3.061],
        [15.41, 2094.71, 2103.19],
        [9.522, 1966.28, 1966.28],
        [8.305, 3.061, 8.0386],
        [12.13, 12.13, 3.061],
        [9.336, 44.41, 44.41],
    ],
    np.float32,
)
I0DIAG = np.array([69585.02, 69585.02, 66666.664], np.float32)

M_MAN = float(MASS.sum())
M_TOT = M_MAN + BASE_MASS + EEF_MASS
K = BASE_MASS + EEF_MASS
BETA = 6.65 * (243.66 / (100000.0 + 243.66))
DCUM = np.stack([DIAGS[j:].sum(0) for j in range(7)], axis=1)  # [a][j]
C1 = (DIAGS.sum(0) + I0DIAG).astype(np.float64)  # [a]

BF = mybir.dt.float16
NPBF = np.float16
SC = 64.0
ADD = mybir.AluOpType.add
MUL = mybir.AluOpType.mult

NCST = 42  # massc 21 | dcum 21


def _const_array() -> np.ndarray:
    row = np.concatenate(
        [
            np.tile(MASS / SC, 3),  # massc[a*7+i] = m_i / SC
            (DCUM / SC).reshape(21),  # dcum[a*7+j] / SC
        ]
    ).astype(NPBF)
    return np.ascontiguousarray(
        np.broadcast_to(row[None, :, None], (P, NCST, F))
    ).reshape(P, NCST * F)


def build_nc():
    nc = bacc.Bacc("TRN2")

    _nb = nc.alloc_sbuf_tensor("const-float32-negbeta", [128, 1], mybir.dt.float32)
    nc.gpsimd.memset(_nb.ap(), -BETA)
    nc.const_aps.aps[(mybir.dt.float32, -BETA)] = _nb.ap()
    nc.all_engine_barrier()

    x_in = nc.dram_tensor("x", [NBLK, P, 63 * F], BF, kind="ExternalInput")
    cst_in = nc.dram_tensor("cst", [P, NCST * F], BF, kind="ExternalInput")
    out_d = nc.dram_tensor("out", [NBLK, P, 42 * F], BF, kind="ExternalOutput")

    V = nc.vector
    G_ = nc.gpsimd

    def emul(E, out, a, b):
        E.scalar_tensor_tensor(out, a, 1.0, b, MUL, MUL)

    def eadd(E, out, a, b):
        E.scalar_tensor_tensor(out, a, 1.0, b, MUL, ADD)

    def esub(E, out, a, b):  # out = a - b
        E.scalar_tensor_tensor(out, b, -1.0, a, MUL, ADD)

    def efma(E, out, a, s, b):  # out = s*a + b
        E.scalar_tensor_tensor(out, a, s, b, MUL, ADD)

    with tile.TileContext(nc) as tc:
        with (
            tc.tile_pool(name="cstp", bufs=1) as cstp,
            tc.tile_pool(name="ioin", bufs=3) as ioin,
            tc.tile_pool(name="io", bufs=2) as io,
            tc.tile_pool(name="wk", bufs=2) as wk,
        ):
            cst = cstp.tile([P, NCST * F], BF, tag="cst")
            nc.scalar.dma_start(cst[:], cst_in[:])
            cv = cst[:].rearrange("p (c f) -> p c f", c=NCST, f=F)
            masscv = cv[:, 0:21, :].rearrange("p (a i) f -> p a i f", a=3, i=7)
            dcum3 = cst[:, 21 * F : 42 * F].rearrange(
                "p (a x) -> p a x", a=3, x=7 * F
            )

            def r2(t, n):  # [P, n, F]
                return t[:].rearrange("p (c f) -> p c f", c=n, f=F)

            def r3(t, a, i):  # [P, a, i, F]
                return t[:].rearrange("p (a i f) -> p a i f", a=a, i=i, f=F)

            def bj(v):  # [P,F] -> [P,7,F] broadcast over j (outermost)
                return v.unsqueeze(1).broadcast_to([P, 7, F])

            def front(b):
                st = {}
                xt = ioin.tile([P, 63 * F], BF, tag="xt")
                nc.sync.dma_start(xt[:, 0 : 42 * F], x_in[b, :, 0 : 42 * F])
                nc.sync.dma_start(xt[:, 42 * F :], x_in[b, :, 42 * F :])
                xv = r3(xt, 9, 7)
                Cv, Ppv, Jv = xv[:, 0:3], xv[:, 3:6], xv[:, 6:9]
                st["xv"], st["Jv"] = xv, Jv

                # DVE: rp, mc ; Pool: rt tree
                rp = wk.tile([P, 21 * F], BF, tag="rp")
                rpv = r3(rp, 3, 7)
                V.tensor_sub(rpv, Cv, Ppv)
                mc = wk.tile([P, 21 * F], BF, tag="mc")
                mcv = r3(mc, 3, 7)
                V.tensor_mul(mcv, masscv, Cv)
                y9 = wk.tile([P, 9 * F], BF, tag="y9")
                y9v = r3(y9, 3, 3)
                V.tensor_add(y9v, mcv[:, :, 0:3, :], mcv[:, :, 3:6, :])
                rt = wk.tile([P, 3 * F], BF, tag="rt")
                rtv = r2(rt, 3)
                V.tensor_add(rtv, y9v[:, :, 0, :], y9v[:, :, 1, :])
                V.tensor_add(rtv, rtv, y9v[:, :, 2, :])
                V.tensor_add(rtv, rtv, mcv[:, :, 6, :])
                st["rtv"] = rtv
                rtm = wk.tile([P, 3 * F], BF, tag="rtm")
                rtmv = r2(rtm, 3)
                V.tensor_scalar_mul(rtmv, rtv, -SC / M_TOT)
                rtm_b4 = rtmv.unsqueeze(2).broadcast_to([P, 3, 7, F])
                st["rtm_b4"] = rtm_b4

                # ACT: rs ; Pool: q ; ACT: g, w
                rs = wk.tile([P, 3 * F], BF, tag="rs")
                rsv = r2(rs, 3)
                nc.scalar.mul(rsv[:, 0:2, :], rtv[:, 0:2, :], SC / M_TOT)
                nc.scalar.activation(
                    rsv[:, 2, :],
                    rtv[:, 2, :],
                    mybir.ActivationFunctionType.Identity,
                    bias=-BETA,
                    scale=SC / M_TOT,
                )
                st["rsv"] = rsv
                q3 = wk.tile([P, 3 * F], BF, tag="q3")
                q3v = r2(q3, 3)
                G_.tensor_mul(q3v, rsv, rsv)
                q = wk.tile([P, F], BF, tag="q")
                qv = q[:]
                G_.tensor_add(qv, q3v[:, 0, :], q3v[:, 1, :])
                G_.tensor_add(qv, qv, q3v[:, 2, :])
                g = wk.tile([P, 3 * F], BF, tag="g")
                gv = r2(g, 3)
                w = wk.tile([P, 3 * F], BF, tag="w")
                wv = r2(w, 3)
                for a in range(3):
                    kc = float(K / C1[a])
                    nc.scalar.activation(
                        gv[:, a, :],
                        qv,
                        mybir.ActivationFunctionType.Identity,
                        bias=1.0,
                        scale=kc,
                    )
                    nc.scalar.mul(wv[:, a, :], rsv[:, a, :], kc)
                st["gv"], st["wv"] = gv, wv

                # DVE: jm ; Pool: rj
                jm = wk.tile([P, 21 * F], BF, tag="jm")
                jmv = r3(jm, 3, 7)
                V.tensor_scalar_mul(
                    r2(jm, 21),
                    xv[:, 6:9].rearrange("p a i f -> p (a i) f"),
                    -SC / M_TOT,
                )
                rjp = wk.tile([P, 21 * F], BF, tag="rjp")
                rjpv = r3(rjp, 3, 7)
                G_.tensor_mul(rjpv, rtm_b4, Jv)  # rjp' = -(rt/M SC)_b * J
                rj = wk.tile([P, 7 * F], BF, tag="rj")
                rjv = r2(rj, 7)
                G_.tensor_add(rjv, rjpv[:, 0], rjpv[:, 1])
                G_.tensor_add(rjv, rjv, rjpv[:, 2])
                st["rj"] = rj

                # DVE: U, suffix, trg
                ut = wk.tile([P, 84 * F], BF, tag="ut")
                Uv = ut[:].rearrange(
                    "p (a d i f) -> p a d i f", a=3, d=4, i=7, f=F
                )
                rp_b = rpv.unsqueeze(2).broadcast_to([P, 3, 3, 7, F])
                mc_b = mcv.unsqueeze(1).broadcast_to([P, 3, 3, 7, F])
                V.tensor_mul(Uv[:, :, 0:3], rp_b, mc_b)
                V.tensor_mul(Uv[:, :, 3], rpv, masscv)
                for j in range(5, -1, -1):
                    V.tensor_add(
                        Uv[:, :, :, j, :], Uv[:, :, :, j, :], Uv[:, :, :, j + 1, :]
                    )
                gd = Uv[:, :, 0:3]  # [P,3(a),3(d),7,F]
                rsuf = Uv[:, :, 3]  # [P,3,7,F]
                st["gd"], st["rsuf"] = gd, rsuf

                st["jmv"] = jmv
                return st

            def back(st, b):
                xv, Jv = st["xv"], st["Jv"]
                gd, rsuf = st["gd"], st["rsuf"]
                rsv, gv, wv = st["rsv"], st["gv"], st["wv"]
                rtm_b4, jmv = st["rtm_b4"], st["jmv"]

                # Pool phase B for this block (issued here so Pool A of the
                # next block runs first in Pool program order)
                trg = wk.tile([P, 7 * F], BF, tag="trg")
                trgv = r2(trg, 7)
                G_.tensor_add(trgv, gd[:, 0, 0], gd[:, 1, 1])
                G_.tensor_add(trgv, trgv, gd[:, 2, 2])
                rrp = wk.tile([P, 21 * F], BF, tag="rrp")
                rrpv = r3(rrp, 3, 7)
                G_.tensor_mul(rrpv, rtm_b4, rsuf)
                vr = wk.tile([P, 21 * F], BF, tag="vr")
                vr3 = vr[:].rearrange("p (a x) -> p a x", a=3, x=7 * F)
                rj_b = st["rj"][:].unsqueeze(1).broadcast_to([P, 3, 7 * F])
                rsuf3 = rsuf.rearrange("p a i f -> p a (i f)")
                G_.tensor_mul(vr3, rj_b, rsuf3)  # vr' = -vr*SC/M
                ja = wk.tile([P, 21 * F], BF, tag="ja")
                jav = r3(ja, 3, 7)
                jb = wk.tile([P, 21 * F], BF, tag="jb")
                jbv = r3(jb, 3, 7)
                for a in range(3):
                    a1_, a2_ = (a + 1) % 3, (a + 2) % 3
                    G_.tensor_mul(jav[:, a], jmv[:, a1_], rsuf[:, a2_])
                    G_.tensor_mul(jbv[:, a], jmv[:, a2_], rsuf[:, a1_])
                G_.tensor_sub(r2(ja, 21), r2(ja, 21), r2(jb, 21))  # jtw'

                # DVE: T, rr', u, a1, hth
                tp = wk.tile([P, 63 * F], BF, tag="tp")
                tpv = tp[:].rearrange(
                    "p (a d j f) -> p a d j f", a=3, d=3, j=7, f=F
                )
                J_b = (
                    Jv.rearrange("p d j f -> p (d j) f")
                    .unsqueeze(1)
                    .broadcast_to([P, 3, 21, F])
                )
                V.tensor_mul(
                    tpv.rearrange("p a d j f -> p a (d j) f"),
                    gd.rearrange("p a d j f -> p a (d j) f"),
                    J_b,
                )
                tt = wk.tile([P, 21 * F], BF, tag="tt")
                ttv = r3(tt, 3, 7)
                V.tensor_add(ttv, tpv[:, :, 0], tpv[:, :, 1])
                V.tensor_add(ttv, ttv, tpv[:, :, 2])

                rr = wk.tile([P, 7 * F], BF, tag="rr")
                rrv = r2(rr, 7)
                V.tensor_add(rrv, rrpv[:, 0], rrpv[:, 1])
                V.tensor_add(rrv, rrv, rrpv[:, 2])
                u7 = wk.tile([P, 7 * F], BF, tag="u7")
                u7v = r2(u7, 7)
                V.tensor_add(u7v, trgv, rrv)  # u = trg - rr/M
                a1 = wk.tile([P, 21 * F], BF, tag="a1")
                a13 = a1[:].rearrange("p (a x) -> p a x", a=3, x=7 * F)
                u_b = u7[:].unsqueeze(1).broadcast_to([P, 3, 7 * F])
                V.tensor_add(a13, dcum3, u_b)

                hth = wk.tile([P, 21 * F], BF, tag="hth")
                hthv = r3(hth, 3, 7)
                hthf = r2(hth, 21)
                V.tensor_mul(
                    hthf, r2(a1, 21), xv[:, 6:9].rearrange("p a i f -> p (a i) f")
                )
                V.tensor_sub(hthf, hthf, r2(tt, 21))
                V.tensor_sub(hthf, hthf, r2(vr, 21))

                # DVE: Y, s, bot, top
                Y = wk.tile([P, 21 * F], BF, tag="Y")
                Yv = r3(Y, 3, 7)
                for a in range(3):
                    V.tensor_scalar_mul(
                        Yv[:, a].rearrange("p i f -> p (i f)"),
                        hthv[:, a].rearrange("p i f -> p (i f)"),
                        float(SC / C1[a]),
                    )
                sp = wk.tile([P, 21 * F], BF, tag="sp")
                spv = r3(sp, 3, 7)
                rs_b4 = rsv.unsqueeze(2).broadcast_to([P, 3, 7, F])
                V.tensor_mul(spv, rs_b4, Yv)
                s7 = wk.tile([P, 7 * F], BF, tag="s7")
                s7v = r2(s7, 7)
                V.tensor_add(s7v, spv[:, 0], spv[:, 1])
                V.tensor_add(s7v, s7v, spv[:, 2])

                outt = io.tile([P, 42 * F], BF, tag="outt")
                outv = r3(outt, 6, 7)

                gy = wk.tile([P, 21 * F], BF, tag="gy")
                gyv = r3(gy, 3, 7)
                t1 = wk.tile([P, 21 * F], BF, tag="t1")
                t1v = r3(t1, 3, 7)
                g_b4 = gv.unsqueeze(2).broadcast_to([P, 3, 7, F])
                w_b4 = wv.unsqueeze(2).broadcast_to([P, 3, 7, F])
                s_b4 = s7v.unsqueeze(1).broadcast_to([P, 3, 7, F])
                V.tensor_mul(gyv, g_b4, Yv)
                V.tensor_mul(t1v, w_b4, s_b4)
                V.tensor_sub(
                    outv[:, 3:6].rearrange("p a j f -> p (a j) f"),
                    r2(t1, 21),
                    r2(gy, 21),
                )  # bot

                ctb = wk.tile([P, 21 * F], BF, tag="ctb")
                ctbv = r3(ctb, 3, 7)
                ctc = wk.tile([P, 21 * F], BF, tag="ctc")
                ctcv = r3(ctc, 3, 7)
                for a in range(3):
                    a1_, a2_ = (a + 1) % 3, (a + 2) % 3
                    V.tensor_mul(ctbv[:, a], bj(rsv[:, a1_, :]), outv[:, 3 + a2_])
                    V.tensor_mul(ctcv[:, a], bj(rsv[:, a2_, :]), outv[:, 3 + a1_])
                V.tensor_sub(r2(ctb, 21), r2(ctb, 21), r2(ctc, 21))
                V.tensor_add(
                    outv[:, 0:3].rearrange("p a j f -> p (a j) f"),
                    r2(ja, 21),
                    r2(ctb, 21),
                )  # top = jtw' + r x bot

                nc.scalar.dma_start(out_d[b], outt[:])

            st_prev = None
            for b in range(NBLK):
                st = front(b)
                if st_prev is not None:
                    back(st_prev, b - 1)
                st_prev = st
            back(st_prev, NBLK - 1)

    nc.compile()
    return nc


_NC_CACHE = None


def _get_nc():
    global _NC_CACHE
    if _NC_CACHE is None:
        _NC_CACHE = build_nc()
    return _NC_CACHE


def _shard_inputs(com_list, link_pose_list, jacobian):
    S = N_SAMPLES * N_HORIZON
    com = np.asarray(com_list, np.float32).reshape(S, 21)
    pos = np.ascontiguousarray(
        np.asarray(link_pose_list, np.float32).reshape(S, 4, 4, 9)[:, 0:3, 3, 0:7]
    ).reshape(S, 21)
    j3 = np.ascontiguousarray(
        np.asarray(jacobian, np.float32).reshape(S, 6, 7)[:, 0:3, :]
    ).reshape(S, 21)
    x = np.concatenate([com, pos, j3], axis=1).astype(NPBF)  # (S, 63)
    x = np.ascontiguousarray(
        x.reshape(N_CORES, NBLK, P, F, 63).transpose(0, 1, 2, 4, 3)
    )  # (cores, NBLK, P, 63, F)
    cst = _const_array()
    return [
        {"x": x[c].reshape(NBLK, P, 63 * F), "cst": cst} for c in range(N_CORES)
    ]


def _gather(results):
    outs = np.stack([r["out"] for r in results])  # (8, NBLK, P, 42F) bf16
    o = outs.reshape(N_CORES, NBLK, P, 42, F).transpose(0, 1, 2, 4, 3)
    return np.ascontiguousarray(o).astype(np.float32).reshape(
        N_SAMPLES, N_HORIZON, 6, 7
    )


def run(com_list, link_pose_list, jacobian, trace=False):
    nc = _get_nc()
    in_maps = _shard_inputs(com_list, link_pose_list, jacobian)
    res = run_bass_kernel_spmd(nc, in_maps, list(range(N_CORES)), trace=trace)
    return _gather(res.results), res


def kernel(com_list, link_pose_list, jacobian):
    out, _ = run(com_list, link_pose_list, jacobian)
    return out


# revision 16
# speedup vs baseline: 1.2077x; 1.0088x over previous
"""Trainium2 Bass kernel for nn_CanadarmJacob (centroidal-dynamics jacobian).

Data-parallel over 8 NeuronCores; per core 32768 flat samples split into
NBLK=4 blocks of [P=128 partitions, F=64 free].  All per-sample quantities
live channel-major ([P, ch*F]) so every vector-op operand has a unit-stride
F-sized last dim -> DVE 2-byte fast modes apply.  Whole pipeline is bf16
scalar_tensor_tensor / tensor_scalar (InstTensorScalarPtr: 4x on DVE,
0.60-eff on Pool); ops are shaped so every access pattern canonicalizes to
<= 3 dims (stt verifier limit).  Only the 63 input floats/sample the
reference actually reads are shipped (com 21, link positions 21, jacobian
rows 0:3), packed host-side into one fused bf16 tensor.

Math (same validated algebra as the fp32 baseline):
  RP = C - P ;  MC = m_i*C ;  U[a,dd,i] = RP[a]*(MC[dd] | m_i)
  G = suffix_j(U) -> G[a,d,j], R[a,j] ;  rt = sum_i MC
  r = rt/M_tot - (0,0,beta) ;  T[a,j] = sum_d G[a,d,j]*J[d,j]
  trG ; rr = sum_a rt[a]R[a,j] ; rj = sum_a rt[a]J[a,j]
  u = trG - rr/M_tot ;  H_th = (DCUM + u)*J - T + (rj_b*R)/M_tot
  J_tw = J_j x R_j
H_s = K r r^T + diag(C1 - K|r|^2) = D + P_m with D = diag(C1) constant and
|P_m|/|D| <= ~4e-3, so first-order Neumann:  H_s^-1 ~= D^-1 - D^-1 P_m D^-1
  Y = Hth/C1 ;  s = sum_a r[a]Y[a,:] ;  q = |r|^2
  bot = w[a]*s - g[a]*Y ,  g = 1 + (K/C1[a]) q ,  w = (K/C1[a]) r[a]
  top = -J_tw/M_tot + r x bot
"""

import os
import sys

for _p in ("/opt/trn_rl_repo", "/root/.axon_site/_ro/trn_rl_repo"):
    if os.path.isdir(_p) and _p not in sys.path:
        sys.path.append(_p)

import numpy as np
import ml_dtypes

import concourse.bass as bass
import concourse.tile as tile
from concourse import bacc, mybir
from concourse.bass_utils import run_bass_kernel_spmd

# ----------------------------------------------------------------- constants
N_SAMPLES, N_HORIZON = 2048, 128
N_CORES = 8
P = 128
F = 64
SPC = N_SAMPLES // N_CORES * N_HORIZON  # 32768
NBLK = SPC // (P * F)  # 4

BASE_MASS, EEF_MASS = 100000.0, 243.66
MASS = np.array([105.98, 105.98, 314.98, 279.2, 105.98, 105.98, 243.66], np.float32)
DIAGS = np.array(
    [
        [12.19, 12.19, 3.061],
        [12.19, 12.19, 3.061],
        [15.41, 2094.71, 2103.19],
        [9.522, 1966.28, 1966.28],
        [8.305, 3.061, 8.0386],
        [12.13, 12.13, 3.061],
        [9.336, 44.41, 44.41],
    ],
    np.float32,
)
I0DIAG = np.array([69585.02, 69585.02, 66666.664], np.float32)

M_MAN = float(MASS.sum())
M_TOT = M_MAN + BASE_MASS + EEF_MASS
K = BASE_MASS + EEF_MASS
BETA = 6.65 * (243.66 / (100000.0 + 243.66))
DCUM = np.stack([DIAGS[j:].sum(0) for j in range(7)], axis=1)  # [a][j]
C1 = (DIAGS.sum(0) + I0DIAG).astype(np.float64)  # [a]

BF = mybir.dt.float16
NPBF = np.float16
SC = 64.0
ADD = mybir.AluOpType.add
MUL = mybir.AluOpType.mult

NCST = 42  # massc 21 | dcum 21


def _const_array() -> np.ndarray:
    row = np.concatenate(
        [
            np.tile(MASS / SC, 3),  # massc[a*7+i] = m_i / SC
            (DCUM / SC).reshape(21),  # dcum[a*7+j] / SC
        ]
    ).astype(NPBF)
    return np.ascontiguousarray(
        np.broadcast_to(row[None, :, None], (P, NCST, F))
    ).reshape(P, NCST * F)


def build_nc():
    nc = bacc.Bacc("TRN2")

    _nb = nc.alloc_sbuf_tensor("const-float32-negbeta", [128, 1], mybir.dt.float32)
    nc.gpsimd.memset(_nb.ap(), -BETA)
    nc.const_aps.aps[(mybir.dt.float32, -BETA)] = _nb.ap()
    nc.all_engine_barrier()

    x_in = nc.dram_tensor("x", [NBLK, P, 63 * F], BF, kind="ExternalInput")
    cst_in = nc.dram_tensor("cst", [P, NCST * F], BF, kind="ExternalInput")
    out_d = nc.dram_tensor("out", [NBLK, P, 42 * F], BF, kind="ExternalOutput")

    V = nc.vector
    G_ = nc.gpsimd

    def emul(E, out, a, b):
        E.scalar_tensor_tensor(out, a, 1.0, b, MUL, MUL)

    def eadd(E, out, a, b):
        E.scalar_tensor_tensor(out, a, 1.0, b, MUL, ADD)

    def esub(E, out, a, b):  # out = a - b
        E.scalar_tensor_tensor(out, b, -1.0, a, MUL, ADD)

    def efma(E, out, a, s, b):  # out = s*a + b
        E.scalar_tensor_tensor(out, a, s, b, MUL, ADD)

    with tile.TileContext(nc) as tc:
        with (
            tc.tile_pool(name="cstp", bufs=1) as cstp,
            tc.tile_pool(name="ioin", bufs=3) as ioin,
            tc.tile_pool(name="io", bufs=2) as io,
            tc.tile_pool(name="wk", bufs=2) as wk,
        ):
            cst = cstp.tile([P, NCST * F], BF, tag="cst")
            nc.scalar.dma_start(cst[:], cst_in[:])
            cv = cst[:].rearrange("p (c f) -> p c f", c=NCST, f=F)
            masscv = cv[:, 0:21, :].rearrange("p (a i) f -> p a i f", a=3, i=7)
            dcum3 = cst[:, 21 * F : 42 * F].rearrange(
                "p (a x) -> p a x", a=3, x=7 * F
            )

            def r2(t, n):  # [P, n, F]
                return t[:].rearrange("p (c f) -> p c f", c=n, f=F)

            def r3(t, a, i):  # [P, a, i, F]
                return t[:].rearrange("p (a i f) -> p a i f", a=a, i=i, f=F)

            def bj(v):  # [P,F] -> [P,7,F] broadcast over j (outermost)
                return v.unsqueeze(1).broadcast_to([P, 7, F])

            def front(b):
                st = {}
                xt = ioin.tile([P, 63 * F], BF, tag="xt")
                nc.sync.dma_start(xt[:, 0 : 42 * F], x_in[b, :, 0 : 42 * F])
                nc.sync.dma_start(xt[:, 42 * F :], x_in[b, :, 42 * F :])
                xv = r3(xt, 9, 7)
                Cv, Ppv, Jv = xv[:, 0:3], xv[:, 3:6], xv[:, 6:9]
                st["xv"], st["Jv"] = xv, Jv

                # DVE: rp, mc ; Pool: rt tree
                rp = wk.tile([P, 21 * F], BF, tag="rp")
                rpv = r3(rp, 3, 7)
                V.tensor_sub(rpv, Cv, Ppv)
                mc = wk.tile([P, 21 * F], BF, tag="mc")
                mcv = r3(mc, 3, 7)
                V.tensor_mul(mcv, masscv, Cv)
                y9 = wk.tile([P, 9 * F], BF, tag="y9")
                y9v = r3(y9, 3, 3)
                V.tensor_add(y9v, mcv[:, :, 0:3, :], mcv[:, :, 3:6, :])
                rt = wk.tile([P, 3 * F], BF, tag="rt")
                rtv = r2(rt, 3)
                V.tensor_add(rtv, y9v[:, :, 0, :], y9v[:, :, 1, :])
                V.tensor_add(rtv, rtv, y9v[:, :, 2, :])
                V.tensor_add(rtv, rtv, mcv[:, :, 6, :])
                st["rtv"] = rtv
                rtm = wk.tile([P, 3 * F], BF, tag="rtm")
                rtmv = r2(rtm, 3)
                V.tensor_scalar_mul(rtmv, rtv, -SC / M_TOT)
                rtm_b4 = rtmv.unsqueeze(2).broadcast_to([P, 3, 7, F])
                st["rtm_b4"] = rtm_b4

                # ACT: rs ; Pool: q ; ACT: g, w
                rs = wk.tile([P, 3 * F], BF, tag="rs")
                rsv = r2(rs, 3)
                nc.scalar.mul(rsv[:, 0:2, :], rtv[:, 0:2, :], SC / M_TOT)
                nc.scalar.activation(
                    rsv[:, 2, :],
                    rtv[:, 2, :],
                    mybir.ActivationFunctionType.Identity,
                    bias=-BETA,
                    scale=SC / M_TOT,
                )
                st["rsv"] = rsv
                w = wk.tile([P, 3 * F], BF, tag="w")
                wv = r2(w, 3)
                for a in range(3):
                    nc.scalar.mul(wv[:, a, :], rsv[:, a, :], float(K / C1[a]))
                st["wv"] = wv

                # DVE: jm ; Pool: rj
                jm = wk.tile([P, 21 * F], BF, tag="jm")
                jmv = r3(jm, 3, 7)
                V.tensor_scalar_mul(
                    r2(jm, 21),
                    xv[:, 6:9].rearrange("p a i f -> p (a i) f"),
                    -SC / M_TOT,
                )

                # DVE: U, suffix, trg
                ut = wk.tile([P, 84 * F], BF, tag="ut")
                Uv = ut[:].rearrange(
                    "p (a d i f) -> p a d i f", a=3, d=4, i=7, f=F
                )
                rp_b = rpv.unsqueeze(2).broadcast_to([P, 3, 3, 7, F])
                mc_b = mcv.unsqueeze(1).broadcast_to([P, 3, 3, 7, F])
                V.tensor_mul(Uv[:, :, 0:3], rp_b, mc_b)
                V.tensor_mul(Uv[:, :, 3], rpv, masscv)
                for j in range(5, -1, -1):
                    V.tensor_add(
                        Uv[:, :, :, j, :], Uv[:, :, :, j, :], Uv[:, :, :, j + 1, :]
                    )
                gd = Uv[:, :, 0:3]  # [P,3(a),3(d),7,F]
                rsuf = Uv[:, :, 3]  # [P,3,7,F]
                st["gd"], st["rsuf"] = gd, rsuf

                st["jmv"] = jmv
                return st

            def back(st, b):
                xv, Jv = st["xv"], st["Jv"]
                gd, rsuf = st["gd"], st["rsuf"]
                rsv, wv = st["rsv"], st["wv"]
                rtm_b4, jmv = st["rtm_b4"], st["jmv"]

                # Pool phase A (rj', q) then phase B for this block
                q3 = wk.tile([P, 3 * F], BF, tag="q3")
                q3v = r2(q3, 3)
                G_.tensor_mul(q3v, rsv, rsv)
                q = wk.tile([P, F], BF, tag="q")
                qv = q[:]
                G_.tensor_add(qv, q3v[:, 0, :], q3v[:, 1, :])
                G_.tensor_add(qv, qv, q3v[:, 2, :])
                g = wk.tile([P, 3 * F], BF, tag="g")
                gv = r2(g, 3)
                for a in range(3):
                    nc.scalar.activation(
                        gv[:, a, :],
                        qv,
                        mybir.ActivationFunctionType.Identity,
                        bias=1.0,
                        scale=float(K / C1[a]),
                    )
                rjp = wk.tile([P, 21 * F], BF, tag="rjp")
                rjpv = r3(rjp, 3, 7)
                G_.tensor_mul(rjpv, rtm_b4, Jv)  # rjp' = -(rt/M SC)_b * J
                rj = wk.tile([P, 7 * F], BF, tag="rj")
                rjv = r2(rj, 7)
                G_.tensor_add(rjv, rjpv[:, 0], rjpv[:, 1])
                G_.tensor_add(rjv, rjv, rjpv[:, 2])

                trg = wk.tile([P, 7 * F], BF, tag="trg")
                trgv = r2(trg, 7)
                G_.tensor_add(trgv, gd[:, 0, 0], gd[:, 1, 1])
                G_.tensor_add(trgv, trgv, gd[:, 2, 2])
                rrp = wk.tile([P, 21 * F], BF, tag="rrp")
                rrpv = r3(rrp, 3, 7)
                G_.tensor_mul(rrpv, rtm_b4, rsuf)
                vr = wk.tile([P, 21 * F], BF, tag="vr")
                vr3 = vr[:].rearrange("p (a x) -> p a x", a=3, x=7 * F)
                rj_b = rj[:].unsqueeze(1).broadcast_to([P, 3, 7 * F])
                rsuf3 = rsuf.rearrange("p a i f -> p a (i f)")
                G_.tensor_mul(vr3, rj_b, rsuf3)  # vr' = -vr*SC/M
                ja = wk.tile([P, 21 * F], BF, tag="ja")
                jav = r3(ja, 3, 7)
                jb = wk.tile([P, 21 * F], BF, tag="jb")
                jbv = r3(jb, 3, 7)
                for a in range(3):
                    a1_, a2_ = (a + 1) % 3, (a + 2) % 3
                    G_.tensor_mul(jav[:, a], jmv[:, a1_], rsuf[:, a2_])
                    G_.tensor_mul(jbv[:, a], jmv[:, a2_], rsuf[:, a1_])
                G_.tensor_sub(r2(ja, 21), r2(ja, 21), r2(jb, 21))  # jtw'

                # DVE: T, rr', u, a1, hth
                tp = wk.tile([P, 63 * F], BF, tag="tp")
                tpv = tp[:].rearrange(
                    "p (a d j f) -> p a d j f", a=3, d=3, j=7, f=F
                )
                J_b = (
                    Jv.rearrange("p d j f -> p (d j) f")
                    .unsqueeze(1)
                    .broadcast_to([P, 3, 21, F])
                )
                V.tensor_mul(
                    tpv.rearrange("p a d j f -> p a (d j) f"),
                    gd.rearrange("p a d j f -> p a (d j) f"),
                    J_b,
                )
                tt = wk.tile([P, 21 * F], BF, tag="tt")
                ttv = r3(tt, 3, 7)
                V.tensor_add(ttv, tpv[:, :, 0], tpv[:, :, 1])
                V.tensor_add(ttv, ttv, tpv[:, :, 2])

                rr = wk.tile([P, 7 * F], BF, tag="rr")
                rrv = r2(rr, 7)
                V.tensor_add(rrv, rrpv[:, 0], rrpv[:, 1])
                V.tensor_add(rrv, rrv, rrpv[:, 2])
                u7 = wk.tile([P, 7 * F], BF, tag="u7")
                u7v = r2(u7, 7)
                V.tensor_add(u7v, trgv, rrv)  # u = trg - rr/M
                a1 = wk.tile([P, 21 * F], BF, tag="a1")
                a13 = a1[:].rearrange("p (a x) -> p a x", a=3, x=7 * F)
                u_b = u7[:].unsqueeze(1).broadcast_to([P, 3, 7 * F])
                V.tensor_add(a13, dcum3, u_b)

                hth = wk.tile([P, 21 * F], BF, tag="hth")
                hthv = r3(hth, 3, 7)
                hthf = r2(hth, 21)
                V.tensor_mul(
                    hthf, r2(a1, 21), xv[:, 6:9].rearrange("p a i f -> p (a i) f")
                )
                V.tensor_sub(hthf, hthf, r2(tt, 21))
                V.tensor_sub(hthf, hthf, r2(vr, 21))

                # DVE: Y, s, bot, top
                Y = wk.tile([P, 21 * F], BF, tag="Y")
                Yv = r3(Y, 3, 7)
                for a in range(3):
                    V.tensor_scalar_mul(
                        Yv[:, a].rearrange("p i f -> p (i f)"),
                        hthv[:, a].rearrange("p i f -> p (i f)"),
                        float(SC / C1[a]),
                    )
                sp = wk.tile([P, 21 * F], BF, tag="sp")
                spv = r3(sp, 3, 7)
                rs_b4 = rsv.unsqueeze(2).broadcast_to([P, 3, 7, F])
                V.tensor_mul(spv, rs_b4, Yv)
                s7 = wk.tile([P, 7 * F], BF, tag="s7")
                s7v = r2(s7, 7)
                V.tensor_add(s7v, spv[:, 0], spv[:, 1])
                V.tensor_add(s7v, s7v, spv[:, 2])

                outt = io.tile([P, 42 * F], BF, tag="outt")
                outv = r3(outt, 6, 7)

                gy = wk.tile([P, 21 * F], BF, tag="gy")
                gyv = r3(gy, 3, 7)
                t1 = wk.tile([P, 21 * F], BF, tag="t1")
                t1v = r3(t1, 3, 7)
                g_b4 = gv.unsqueeze(2).broadcast_to([P, 3, 7, F])
                w_b4 = wv.unsqueeze(2).broadcast_to([P, 3, 7, F])
                s_b4 = s7v.unsqueeze(1).broadcast_to([P, 3, 7, F])
                V.tensor_mul(gyv, g_b4, Yv)
                V.tensor_mul(t1v, w_b4, s_b4)
                V.tensor_sub(
                    outv[:, 3:6].rearrange("p a j f -> p (a j) f"),
                    r2(t1, 21),
                    r2(gy, 21),
                )  # bot
                nc.scalar.dma_start(
                    out_d[b, :, 21 * F :], outt[:, 21 * F :]
                )

                ctb = wk.tile([P, 21 * F], BF, tag="ctb")
                ctbv = r3(ctb, 3, 7)
                ctc = wk.tile([P, 21 * F], BF, tag="ctc")
                ctcv = r3(ctc, 3, 7)
                for a in range(3):
                    a1_, a2_ = (a + 1) % 3, (a + 2) % 3
                    V.tensor_mul(ctbv[:, a], bj(rsv[:, a1_, :]), outv[:, 3 + a2_])
                    V.tensor_mul(ctcv[:, a], bj(rsv[:, a2_, :]), outv[:, 3 + a1_])
                V.tensor_sub(r2(ctb, 21), r2(ctb, 21), r2(ctc, 21))
                V.tensor_add(
                    outv[:, 0:3].rearrange("p a j f -> p (a j) f"),
                    r2(ja, 21),
                    r2(ctb, 21),
                )  # top = jtw' + r x bot

                nc.scalar.dma_start(out_d[b, :, 0 : 21 * F], outt[:, 0 : 21 * F])

            st_prev = None
            for b in range(NBLK):
                st = front(b)
                if st_prev is not None:
                    back(st_prev, b - 1)
                st_prev = st
            back(st_prev, NBLK - 1)

    nc.compile()
    return nc


_NC_CACHE = None


def _get_nc():
    global _NC_CACHE
    if _NC_CACHE is None:
        _NC_CACHE = build_nc()
    return _NC_CACHE


def _shard_inputs(com_list, link_pose_list, jacobian):
    S = N_SAMPLES * N_HORIZON
    com = np.asarray(com_list, np.float32).reshape(S, 21)
    pos = np.ascontiguousarray(
        np.asarray(link_pose_list, np.float32).reshape(S, 4, 4, 9)[:, 0:3, 3, 0:7]
    ).reshape(S, 21)
    j3 = np.ascontiguousarray(
        np.asarray(jacobian, np.float32).reshape(S, 6, 7)[:, 0:3, :]
    ).reshape(S, 21)
    x = np.concatenate([com, pos, j3], axis=1).astype(NPBF)  # (S, 63)
    x = np.ascontiguousarray(
        x.reshape(N_CORES, NBLK, P, F, 63).transpose(0, 1, 2, 4, 3)
    )  # (cores, NBLK, P, 63, F)
    cst = _const_array()
    return [
        {"x": x[c].reshape(NBLK, P, 63 * F), "cst": cst} for c in range(N_CORES)
    ]


def _gather(results):
    outs = np.stack([r["out"] for r in results])  # (8, NBLK, P, 42F) bf16
    o = outs.reshape(N_CORES, NBLK, P, 42, F).transpose(0, 1, 2, 4, 3)
    return np.ascontiguousarray(o).astype(np.float32).reshape(
        N_SAMPLES, N_HORIZON, 6, 7
    )


def run(com_list, link_pose_list, jacobian, trace=False):
    nc = _get_nc()
    in_maps = _shard_inputs(com_list, link_pose_list, jacobian)
    res = run_bass_kernel_spmd(nc, in_maps, list(range(N_CORES)), trace=trace)
    return _gather(res.results), res


def kernel(com_list, link_pose_list, jacobian):
    out, _ = run(com_list, link_pose_list, jacobian)
    return out


# revision 17
# speedup vs baseline: 1.2242x; 1.0137x over previous
"""Trainium2 Bass kernel for nn_CanadarmJacob (centroidal-dynamics jacobian).

Data-parallel over 8 NeuronCores; per core 32768 flat samples split into
NBLK=4 blocks of [P=128 partitions, F=64 free].  All per-sample quantities
live channel-major ([P, ch*F]) so every vector-op operand has a unit-stride
F-sized last dim -> DVE 2-byte fast modes apply.  Whole pipeline is bf16
scalar_tensor_tensor / tensor_scalar (InstTensorScalarPtr: 4x on DVE,
0.60-eff on Pool); ops are shaped so every access pattern canonicalizes to
<= 3 dims (stt verifier limit).  Only the 63 input floats/sample the
reference actually reads are shipped (com 21, link positions 21, jacobian
rows 0:3), packed host-side into one fused bf16 tensor.

Math (same validated algebra as the fp32 baseline):
  RP = C - P ;  MC = m_i*C ;  U[a,dd,i] = RP[a]*(MC[dd] | m_i)
  G = suffix_j(U) -> G[a,d,j], R[a,j] ;  rt = sum_i MC
  r = rt/M_tot - (0,0,beta) ;  T[a,j] = sum_d G[a,d,j]*J[d,j]
  trG ; rr = sum_a rt[a]R[a,j] ; rj = sum_a rt[a]J[a,j]
  u = trG - rr/M_tot ;  H_th = (DCUM + u)*J - T + (rj_b*R)/M_tot
  J_tw = J_j x R_j
H_s = K r r^T + diag(C1 - K|r|^2) = D + P_m with D = diag(C1) constant and
|P_m|/|D| <= ~4e-3, so first-order Neumann:  H_s^-1 ~= D^-1 - D^-1 P_m D^-1
  Y = Hth/C1 ;  s = sum_a r[a]Y[a,:] ;  q = |r|^2
  bot = w[a]*s - g[a]*Y ,  g = 1 + (K/C1[a]) q ,  w = (K/C1[a]) r[a]
  top = -J_tw/M_tot + r x bot
"""

import os
import sys

for _p in ("/opt/trn_rl_repo", "/root/.axon_site/_ro/trn_rl_repo"):
    if os.path.isdir(_p) and _p not in sys.path:
        sys.path.append(_p)

import numpy as np
import ml_dtypes

import concourse.bass as bass
import concourse.tile as tile
from concourse import bacc, mybir
from concourse.bass_utils import run_bass_kernel_spmd

# ----------------------------------------------------------------- constants
N_SAMPLES, N_HORIZON = 2048, 128
N_CORES = 8
P = 128
F = 64
SPC = N_SAMPLES // N_CORES * N_HORIZON  # 32768
NBLK = SPC // (P * F)  # 4

BASE_MASS, EEF_MASS = 100000.0, 243.66
MASS = np.array([105.98, 105.98, 314.98, 279.2, 105.98, 105.98, 243.66], np.float32)
DIAGS = np.array(
    [
        [12.19, 12.19, 3.061],
        [12.19, 12.19, 3.061],
        [15.41, 2094.71, 2103.19],
        [9.522, 1966.28, 1966.28],
        [8.305, 3.061, 8.0386],
        [12.13, 12.13, 3.061],
        [9.336, 44.41, 44.41],
    ],
    np.float32,
)
I0DIAG = np.array([69585.02, 69585.02, 66666.664], np.float32)

M_MAN = float(MASS.sum())
M_TOT = M_MAN + BASE_MASS + EEF_MASS
K = BASE_MASS + EEF_MASS
BETA = 6.65 * (243.66 / (100000.0 + 243.66))
DCUM = np.stack([DIAGS[j:].sum(0) for j in range(7)], axis=1)  # [a][j]
C1 = (DIAGS.sum(0) + I0DIAG).astype(np.float64)  # [a]

BF = mybir.dt.float16
NPBF = np.float16
SC = 64.0
ADD = mybir.AluOpType.add
MUL = mybir.AluOpType.mult

NCST = 42  # massc 21 | dcum 21


def _const_array() -> np.ndarray:
    row = np.concatenate(
        [
            np.tile(MASS / SC, 3),  # massc[a*7+i] = m_i / SC
            (DCUM / SC).reshape(21),  # dcum[a*7+j] / SC
        ]
    ).astype(NPBF)
    return np.ascontiguousarray(
        np.broadcast_to(row[None, :, None], (P, NCST, F))
    ).reshape(P, NCST * F)


def build_nc():
    nc = bacc.Bacc("TRN2")

    _nb = nc.alloc_sbuf_tensor("const-float32-negbeta", [128, 1], mybir.dt.float32)
    nc.gpsimd.memset(_nb.ap(), -BETA)
    nc.const_aps.aps[(mybir.dt.float32, -BETA)] = _nb.ap()
    nc.all_engine_barrier()

    x_in = nc.dram_tensor("x", [NBLK, P, 63 * F], BF, kind="ExternalInput")
    cst_in = nc.dram_tensor("cst", [P, NCST * F], BF, kind="ExternalInput")
    out_d = nc.dram_tensor("out", [NBLK, P, 42 * F], BF, kind="ExternalOutput")

    V = nc.vector
    G_ = nc.gpsimd

    def emul(E, out, a, b):
        E.scalar_tensor_tensor(out, a, 1.0, b, MUL, MUL)

    def eadd(E, out, a, b):
        E.scalar_tensor_tensor(out, a, 1.0, b, MUL, ADD)

    def esub(E, out, a, b):  # out = a - b
        E.scalar_tensor_tensor(out, b, -1.0, a, MUL, ADD)

    def efma(E, out, a, s, b):  # out = s*a + b
        E.scalar_tensor_tensor(out, a, s, b, MUL, ADD)

    with tile.TileContext(nc) as tc:
        with (
            tc.tile_pool(name="cstp", bufs=1) as cstp,
            tc.tile_pool(name="ioin", bufs=3) as ioin,
            tc.tile_pool(name="io", bufs=2) as io,
            tc.tile_pool(name="wk", bufs=2) as wk,
        ):
            cst = cstp.tile([P, NCST * F], BF, tag="cst")
            nc.scalar.dma_start(cst[:], cst_in[:])
            cv = cst[:].rearrange("p (c f) -> p c f", c=NCST, f=F)
            masscv = cv[:, 0:21, :].rearrange("p (a i) f -> p a i f", a=3, i=7)
            dcum3 = cst[:, 21 * F : 42 * F].rearrange(
                "p (a x) -> p a x", a=3, x=7 * F
            )

            def r2(t, n):  # [P, n, F]
                return t[:].rearrange("p (c f) -> p c f", c=n, f=F)

            def r3(t, a, i):  # [P, a, i, F]
                return t[:].rearrange("p (a i f) -> p a i f", a=a, i=i, f=F)

            def bj(v):  # [P,F] -> [P,7,F] broadcast over j (outermost)
                return v.unsqueeze(1).broadcast_to([P, 7, F])

            def front(b):
                st = {}
                xt = ioin.tile([P, 63 * F], BF, tag="xt")
                nc.sync.dma_start(xt[:, 0 : 42 * F], x_in[b, :, 0 : 42 * F])
                nc.sync.dma_start(xt[:, 42 * F :], x_in[b, :, 42 * F :])
                xv = r3(xt, 9, 7)
                Cv, Ppv, Jv = xv[:, 0:3], xv[:, 3:6], xv[:, 6:9]
                st["xv"], st["Jv"] = xv, Jv

                # DVE: rp, mc ; Pool: rt tree
                rp = wk.tile([P, 21 * F], BF, tag="rp")
                rpv = r3(rp, 3, 7)
                V.tensor_sub(rpv, Cv, Ppv)
                mc = wk.tile([P, 21 * F], BF, tag="mc")
                mcv = r3(mc, 3, 7)
                V.tensor_mul(mcv, masscv, Cv)
                y9 = wk.tile([P, 9 * F], BF, tag="y9")
                y9v = r3(y9, 3, 3)
                V.tensor_add(y9v, mcv[:, :, 0:3, :], mcv[:, :, 3:6, :])
                rt = wk.tile([P, 3 * F], BF, tag="rt")
                rtv = r2(rt, 3)
                V.tensor_add(rtv, y9v[:, :, 0, :], y9v[:, :, 1, :])
                V.tensor_add(rtv, rtv, y9v[:, :, 2, :])
                V.tensor_add(rtv, rtv, mcv[:, :, 6, :])
                st["rtv"] = rtv
                rtm = wk.tile([P, 3 * F], BF, tag="rtm")
                rtmv = r2(rtm, 3)
                nc.scalar.mul(rtmv, rtv, -SC / M_TOT)
                rtm_b4 = rtmv.unsqueeze(2).broadcast_to([P, 3, 7, F])
                st["rtm_b4"] = rtm_b4

                # ACT: rs ; Pool: q ; ACT: g, w
                rs = wk.tile([P, 3 * F], BF, tag="rs")
                rsv = r2(rs, 3)
                nc.scalar.mul(rsv[:, 0:2, :], rtv[:, 0:2, :], SC / M_TOT)
                nc.scalar.activation(
                    rsv[:, 2, :],
                    rtv[:, 2, :],
                    mybir.ActivationFunctionType.Identity,
                    bias=-BETA,
                    scale=SC / M_TOT,
                )
                st["rsv"] = rsv
                w = wk.tile([P, 3 * F], BF, tag="w")
                wv = r2(w, 3)
                for a in range(3):
                    nc.scalar.mul(wv[:, a, :], rsv[:, a, :], float(K / C1[a]))
                st["wv"] = wv

                # ACT: jm
                jm = wk.tile([P, 21 * F], BF, tag="jm")
                jmv = r3(jm, 3, 7)
                nc.scalar.mul(
                    r2(jm, 21),
                    xv[:, 6:9].rearrange("p a i f -> p (a i) f"),
                    -SC / M_TOT,
                )

                # DVE: U, suffix, trg
                ut = wk.tile([P, 84 * F], BF, tag="ut")
                Uv = ut[:].rearrange(
                    "p (a d i f) -> p a d i f", a=3, d=4, i=7, f=F
                )
                rp_b = rpv.unsqueeze(2).broadcast_to([P, 3, 3, 7, F])
                mc_b = mcv.unsqueeze(1).broadcast_to([P, 3, 3, 7, F])
                V.tensor_mul(Uv[:, :, 0:3], rp_b, mc_b)
                V.tensor_mul(Uv[:, :, 3], rpv, masscv)
                for j in range(5, -1, -1):
                    V.tensor_add(
                        Uv[:, :, :, j, :], Uv[:, :, :, j, :], Uv[:, :, :, j + 1, :]
                    )
                gd = Uv[:, :, 0:3]  # [P,3(a),3(d),7,F]
                rsuf = Uv[:, :, 3]  # [P,3,7,F]
                st["gd"], st["rsuf"] = gd, rsuf

                st["jmv"] = jmv
                return st

            def back(st, b):
                xv, Jv = st["xv"], st["Jv"]
                gd, rsuf = st["gd"], st["rsuf"]
                rsv, wv = st["rsv"], st["wv"]
                rtm_b4, jmv = st["rtm_b4"], st["jmv"]

                # ACT: q3 (square), q tree on Pool, g on ACT
                q3 = wk.tile([P, 3 * F], BF, tag="q3")
                q3v = r2(q3, 3)
                nc.scalar.square(q3v, rsv)
                q = wk.tile([P, F], BF, tag="q")
                qv = q[:]
                G_.tensor_add(qv, q3v[:, 0, :], q3v[:, 1, :])
                G_.tensor_add(qv, qv, q3v[:, 2, :])
                g = wk.tile([P, 3 * F], BF, tag="g")
                gv = r2(g, 3)
                for a in range(3):
                    nc.scalar.activation(
                        gv[:, a, :],
                        qv,
                        mybir.ActivationFunctionType.Identity,
                        bias=1.0,
                        scale=float(K / C1[a]),
                    )

                # Pool phase B, earliest-deadline-first for DVE consumers
                rrp = wk.tile([P, 21 * F], BF, tag="rrp")
                rrpv = r3(rrp, 3, 7)
                G_.tensor_mul(rrpv, rtm_b4, rsuf)
                trg = wk.tile([P, 7 * F], BF, tag="trg")
                trgv = r2(trg, 7)
                G_.tensor_add(trgv, gd[:, 0, 0], gd[:, 1, 1])
                G_.tensor_add(trgv, trgv, gd[:, 2, 2])
                rjp = wk.tile([P, 21 * F], BF, tag="rjp")
                rjpv = r3(rjp, 3, 7)
                G_.tensor_mul(rjpv, rtm_b4, Jv)  # rjp' = -(rt/M SC)_b * J
                rj = wk.tile([P, 7 * F], BF, tag="rj")
                rjv = r2(rj, 7)
                G_.tensor_add(rjv, rjpv[:, 0], rjpv[:, 1])
                G_.tensor_add(rjv, rjv, rjpv[:, 2])
                vr = wk.tile([P, 21 * F], BF, tag="vr")
                vr3 = vr[:].rearrange("p (a x) -> p a x", a=3, x=7 * F)
                rj_b = rj[:].unsqueeze(1).broadcast_to([P, 3, 7 * F])
                rsuf3 = rsuf.rearrange("p a i f -> p a (i f)")
                G_.tensor_mul(vr3, rj_b, rsuf3)  # vr' = -vr*SC/M
                ja = wk.tile([P, 21 * F], BF, tag="ja")
                jav = r3(ja, 3, 7)
                jb = wk.tile([P, 21 * F], BF, tag="jb")
                jbv = r3(jb, 3, 7)
                for a in range(3):
                    a1_, a2_ = (a + 1) % 3, (a + 2) % 3
                    G_.tensor_mul(jav[:, a], jmv[:, a1_], rsuf[:, a2_])
                    G_.tensor_mul(jbv[:, a], jmv[:, a2_], rsuf[:, a1_])
                G_.tensor_sub(r2(ja, 21), r2(ja, 21), r2(jb, 21))  # jtw'

                # DVE: T, rr', u, a1, hth
                tp = wk.tile([P, 63 * F], BF, tag="tp")
                tpv = tp[:].rearrange(
                    "p (a d j f) -> p a d j f", a=3, d=3, j=7, f=F
                )
                J_b = (
                    Jv.rearrange("p d j f -> p (d j) f")
                    .unsqueeze(1)
                    .broadcast_to([P, 3, 21, F])
                )
                V.tensor_mul(
                    tpv.rearrange("p a d j f -> p a (d j) f"),
                    gd.rearrange("p a d j f -> p a (d j) f"),
                    J_b,
                )
                tt = wk.tile([P, 21 * F], BF, tag="tt")
                ttv = r3(tt, 3, 7)
                V.tensor_add(ttv, tpv[:, :, 0], tpv[:, :, 1])
                V.tensor_add(ttv, ttv, tpv[:, :, 2])

                rr = wk.tile([P, 7 * F], BF, tag="rr")
                rrv = r2(rr, 7)
                V.tensor_add(rrv, rrpv[:, 0], rrpv[:, 1])
                V.tensor_add(rrv, rrv, rrpv[:, 2])
                u7 = wk.tile([P, 7 * F], BF, tag="u7")
                u7v = r2(u7, 7)
                V.tensor_add(u7v, trgv, rrv)  # u = trg - rr/M
                a1 = wk.tile([P, 21 * F], BF, tag="a1")
                a13 = a1[:].rearrange("p (a x) -> p a x", a=3, x=7 * F)
                u_b = u7[:].unsqueeze(1).broadcast_to([P, 3, 7 * F])
                V.tensor_add(a13, dcum3, u_b)

                hth = wk.tile([P, 21 * F], BF, tag="hth")
                hthv = r3(hth, 3, 7)
                hthf = r2(hth, 21)
                V.tensor_mul(
                    hthf, r2(a1, 21), xv[:, 6:9].rearrange("p a i f -> p (a i) f")
                )
                V.tensor_sub(hthf, hthf, r2(tt, 21))
                V.tensor_sub(hthf, hthf, r2(vr, 21))

                # DVE: Y, s, bot, top
                Y = wk.tile([P, 21 * F], BF, tag="Y")
                Yv = r3(Y, 3, 7)
                for a in range(3):
                    nc.scalar.mul(
                        Yv[:, a].rearrange("p i f -> p (i f)"),
                        hthv[:, a].rearrange("p i f -> p (i f)"),
                        float(SC / C1[a]),
                    )
                sp = wk.tile([P, 21 * F], BF, tag="sp")
                spv = r3(sp, 3, 7)
                rs_b4 = rsv.unsqueeze(2).broadcast_to([P, 3, 7, F])
                V.tensor_mul(spv, rs_b4, Yv)
                s7 = wk.tile([P, 7 * F], BF, tag="s7")
                s7v = r2(s7, 7)
                V.tensor_add(s7v, spv[:, 0], spv[:, 1])
                V.tensor_add(s7v, s7v, spv[:, 2])

                outt = io.tile([P, 42 * F], BF, tag="outt")
                outv = r3(outt, 6, 7)

                gy = wk.tile([P, 21 * F], BF, tag="gy")
                gyv = r3(gy, 3, 7)
                t1 = wk.tile([P, 21 * F], BF, tag="t1")
                t1v = r3(t1, 3, 7)
                g_b4 = gv.unsqueeze(2).broadcast_to([P, 3, 7, F])
                w_b4 = wv.unsqueeze(2).broadcast_to([P, 3, 7, F])
                s_b4 = s7v.unsqueeze(1).broadcast_to([P, 3, 7, F])
                V.tensor_mul(gyv, g_b4, Yv)
                V.tensor_mul(t1v, w_b4, s_b4)
                V.tensor_sub(
                    outv[:, 3:6].rearrange("p a j f -> p (a j) f"),
                    r2(t1, 21),
                    r2(gy, 21),
                )  # bot
                nc.scalar.dma_start(
                    out_d[b, :, 21 * F :], outt[:, 21 * F :]
                )

                ctb = wk.tile([P, 21 * F], BF, tag="ctb")
                ctbv = r3(ctb, 3, 7)
                ctc = wk.tile([P, 21 * F], BF, tag="ctc")
                ctcv = r3(ctc, 3, 7)
                for a in range(3):
                    a1_, a2_ = (a + 1) % 3, (a + 2) % 3
                    V.tensor_mul(ctbv[:, a], bj(rsv[:, a1_, :]), outv[:, 3 + a2_])
                    V.tensor_mul(ctcv[:, a], bj(rsv[:, a2_, :]), outv[:, 3 + a1_])
                V.tensor_sub(r2(ctb, 21), r2(ctb, 21), r2(ctc, 21))
                V.tensor_add(
                    outv[:, 0:3].rearrange("p a j f -> p (a j) f"),
                    r2(ja, 21),
                    r2(ctb, 21),
                )  # top = jtw' + r x bot

                nc.scalar.dma_start(out_d[b, :, 0 : 21 * F], outt[:, 0 : 21 * F])

            st_prev = None
            for b in range(NBLK):
                st = front(b)
                if st_prev is not None:
                    back(st_prev, b - 1)
                st_prev = st
            back(st_prev, NBLK - 1)

    nc.compile()
    return nc


_NC_CACHE = None


def _get_nc():
    global _NC_CACHE
    if _NC_CACHE is None:
        _NC_CACHE = build_nc()
    return _NC_CACHE


def _shard_inputs(com_list, link_pose_list, jacobian):
    S = N_SAMPLES * N_HORIZON
    com = np.asarray(com_list, np.float32).reshape(S, 21)
    pos = np.ascontiguousarray(
        np.asarray(link_pose_list, np.float32).reshape(S, 4, 4, 9)[:, 0:3, 3, 0:7]
    ).reshape(S, 21)
    j3 = np.ascontiguousarray(
        np.asarray(jacobian, np.float32).reshape(S, 6, 7)[:, 0:3, :]
    ).reshape(S, 21)
    x = np.concatenate([com, pos, j3], axis=1).astype(NPBF)  # (S, 63)
    x = np.ascontiguousarray(
        x.reshape(N_CORES, NBLK, P, F, 63).transpose(0, 1, 2, 4, 3)
    )  # (cores, NBLK, P, 63, F)
    cst = _const_array()
    return [
        {"x": x[c].reshape(NBLK, P, 63 * F), "cst": cst} for c in range(N_CORES)
    ]


def _gather(results):
    outs = np.stack([r["out"] for r in results])  # (8, NBLK, P, 42F) bf16
    o = outs.reshape(N_CORES, NBLK, P, 42, F).transpose(0, 1, 2, 4, 3)
    return np.ascontiguousarray(o).astype(np.float32).reshape(
        N_SAMPLES, N_HORIZON, 6, 7
    )


def run(com_list, link_pose_list, jacobian, trace=False):
    nc = _get_nc()
    in_maps = _shard_inputs(com_list, link_pose_list, jacobian)
    res = run_bass_kernel_spmd(nc, in_maps, list(range(N_CORES)), trace=trace)
    return _gather(res.results), res


def kernel(com_list, link_pose_list, jacobian):
    out, _ = run(com_list, link_pose_list, jacobian)
    return out
